# revision 3
# baseline (speedup 1.0000x reference)
"""Butterfly (Givens) rotation network on TRN2, 8 NeuronCores.

Algorithm
---------
x: (8192, 4096) f32. 12 butterfly layers; layer l rotates pairs of features
differing in bit l of the feature index. Split into two linear stages:

  Stage A = layers 0-6: features mix only within 128-wide blocks b (bits 0-6)
            -> per-block 128x128 matrix A_b.
  Stage B = layers 7-11: features mix only across blocks at fixed within-block
            position p (bits 7-11) -> per-p 32x32 matrix B_p; grouping 4
            consecutive p per 128-partition tile gives block-diag 128x128.

Per 128-row tile (rows on partitions), all on the TensorEngine:
  pass1: per block b: PE-transpose x_b -> xT_b [f',r] (PSUM->SBUF copy),
         MM out[r,fo] = sum_f' xT_b[f',r] * A_bT[f',fo]  (lhsT=xT_b, rhs=A_bT)
         scatter-copy PSUM->SBUF into Y with f~ = (p//4)*128 + (p%4)*32 + b.
  pass2: per f~-tile t: PE-transpose Y_t -> z [f~',r],
         MM out[r,n] = sum z[f~',r] * BDT_t[f~',n], scatter-copy to natural
         feature order, DMA out.

Sharding: data-parallel over rows, 1024 rows/core; matrices replicated.
"""

import math
import numpy as np

DIM = 4096
NL = 12
NB = 32          # 128-wide feature blocks
ROWS = 8192
NCORES = 8
RPC = ROWS // NCORES     # rows per core
NT = RPC // 128          # 128-row tiles per core


# ---------------------------------------------------------------- host math

def _butterfly_np(x, angles):
    """float64 numpy copy of the reference butterfly."""
    x = np.asarray(x, np.float64)
    angles = np.asarray(angles, np.float64)
    B, d = x.shape
    for l in range(angles.shape[0]):
        stride = 2 ** l
        nblocks = d // (2 * stride)
        xr = x.reshape(B, nblocks, 2, stride)
        c = np.cos(angles[l]).reshape(nblocks, stride)
        s = np.sin(angles[l]).reshape(nblocks, stride)
        xi = xr[:, :, 0, :].copy()
        xj = xr[:, :, 1, :].copy()
        x = np.stack([c * xi + s * xj, -s * xi + c * xj], axis=2).reshape(B, d)
    return x


def _build_mats(angles):
    """Returns (amats, bmats) each [128, 4096] f32 in SBUF-ready layout."""
    angles = np.asarray(angles, np.float64)
    amats = np.zeros((128, DIM), np.float64)
    for b in range(NB):
        # A_bT[f_in, f_out]: butterfly of identity rows = F^T for this block
        amats[:, 128 * b:128 * b + 128] = _butterfly_np(
            np.eye(128), angles[0:7, 64 * b:64 * b + 64])
    bmats = np.zeros((128, DIM), np.float64)
    for t in range(32):
        for pl in range(4):
            p = 4 * t + pl
            BpT = _butterfly_np(np.eye(32), angles[7:12, p::128])
            bmats[32 * pl:32 * pl + 32, 128 * t + 32 * pl:128 * t + 32 * pl + 32] = BpT
    return amats.astype(np.float32), bmats.astype(np.float32)


# ---------------------------------------------------------------- bass kernel

def _emit_kernel(ctx, tc, out, x, amats, bmats, ident):
    import concourse.bass as bass
    import concourse.mybir as mybir

    nc = tc.nc
    f32 = mybir.dt.float32

    consts = ctx.enter_context(tc.tile_pool(name="consts", bufs=1))
    xin = ctx.enter_context(tc.tile_pool(name="xin", bufs=3))
    ystage = ctx.enter_context(tc.tile_pool(name="ystage", bufs=3))
    ostage = ctx.enter_context(tc.tile_pool(name="ostage", bufs=3))
    sbst = ctx.enter_context(tc.tile_pool(name="sbst", bufs=6))
    psA = ctx.enter_context(tc.tile_pool(name="psA", bufs=4, space="PSUM"))
    psB = ctx.enter_context(tc.tile_pool(name="psB", bufs=4, space="PSUM"))

    am = consts.tile([128, DIM], f32, tag="amats")
    bm = consts.tile([128, DIM], f32, tag="bmats")
    idt = consts.tile([128, 128], f32, tag="ident")
    nc.sync.dma_start(idt[:], ident[:])

    # Greedy least-loaded assignment of PSUM->SBUF copies to DVE/ACT,
    # using measured per-copy costs (ns) for [128,512] fp32 from PSUM.
    load = {"dve": 0.0, "act": 0.0}
    cost = {("dve", "plain"): 685, ("dve", "scatter"): 700,
            ("act", "plain"): 570, ("act", "scatter"): 1127}

    def copy(dst, src, kind="plain"):
        eng = min(("dve", "act"), key=lambda e: load[e] + cost[(e, kind)])
        load[eng] += cost[(eng, kind)]
        (nc.vector.tensor_copy if eng == "dve" else nc.scalar.copy)(dst, src)

    for i in range(NT):
        xt = xin.tile([128, DIM], f32, tag="xt")
        if i == 0:
            # first tile: fine-grained x/amats chunk interleave so the very
            # first transposes and stage-A matmuls start as early as possible
            for c in range(8):
                nc.sync.dma_start(xt[:, 512 * c:512 * (c + 1)],
                                  x[0:128, 512 * c:512 * (c + 1)])
                nc.sync.dma_start(am[:, 512 * c:512 * (c + 1)],
                                  amats[:, 512 * c:512 * (c + 1)])
        else:
            nc.sync.dma_start(xt[:], x[128 * i:128 * (i + 1), :])
        Y = ystage.tile([128, DIM], f32, tag="Y")

        for g in range(8):           # groups of 4 feature blocks
            pt = psA.tile([128, 512], f32, tag="ptA")
            for j in range(4):
                b = 4 * g + j
                nc.tensor.transpose(
                    pt[:, 128 * j:128 * (j + 1)],
                    xt[:, 128 * b:128 * (b + 1)], idt[:])
            xT4 = sbst.tile([128, 512], f32, tag="xT4")
            copy(xT4[:], pt[:])
            pm = psB.tile([128, 512], f32, tag="pmA")
            for j in range(4):
                b = 4 * g + j
                nc.tensor.matmul(
                    pm[:, 128 * j:128 * (j + 1)],
                    xT4[:, 128 * j:128 * (j + 1)],
                    am[:, 128 * b:128 * (b + 1)],
                    start=True, stop=True)
            # scatter into Y: dest f~ = t*128 + pl*32 + (4g+j), src = j*128 + 4t + pl
            src = pm[:].rearrange("r (j t pl) -> r j t pl", j=4, t=32, pl=4)
            dst = Y[:].rearrange(
                "r (t pl g j) -> r g j t pl", t=32, pl=4, g=8, j=4)[:, g]
            copy(dst, src, kind="scatter")

        if i == 0:
            for c in range(8):
                nc.sync.dma_start(bm[:, 512 * c:512 * (c + 1)],
                                  bmats[:, 512 * c:512 * (c + 1)])
        O = ostage.tile([128, DIM], f32, tag="O")
        for g in range(8):           # groups of 4 f~ tiles
            pt = psA.tile([128, 512], f32, tag="ptA")
            for j in range(4):
                t = 4 * g + j
                nc.tensor.transpose(
                    pt[:, 128 * j:128 * (j + 1)],
                    Y[:, 128 * t:128 * (t + 1)], idt[:])
            z4 = sbst.tile([128, 512], f32, tag="xT4")
            copy(z4[:], pt[:])
            pm = psB.tile([128, 512], f32, tag="pmA")
            for j in range(4):
                t = 4 * g + j
                nc.tensor.matmul(
                    pm[:, 128 * j:128 * (j + 1)],
                    z4[:, 128 * j:128 * (j + 1)],
                    bm[:, 128 * t:128 * (t + 1)],
                    start=True, stop=True)
            # scatter to natural order: dest f = b*128 + 4t + pl = b*128 + 16g + 4j + pl
            src = pm[:].rearrange("r (j pl b) -> r j pl b", j=4, pl=4, b=32)
            dst = O[:].rearrange(
                "r (b g j pl) -> r g j pl b", b=32, g=8, j=4, pl=4)[:, g]
            copy(dst, src, kind="scatter")

        nc.sync.dma_start(out[128 * i:128 * (i + 1), :], O[:])


def _emit_kernel_v2(ctx, tc, out, x, amats, bmats, ident):
    """f32r weights-stationary variant: super-tiles of 256 rows, stage
    matmuls lhsT=matrix rhs=data at N=256 (f32r streams 1 cyc/row vs 4 for
    fp32), data kept feature-major between stages, f32r transposes (1.5
    cyc/row) for all shuffles after the first exact fp32 transpose."""
    import concourse.mybir as mybir

    nc = tc.nc
    f32 = mybir.dt.float32
    f32r = mybir.dt.float32r

    consts = ctx.enter_context(tc.tile_pool(name="consts", bufs=1))
    mstage = ctx.enter_context(tc.tile_pool(name="mstage", bufs=1))
    xin = ctx.enter_context(tc.tile_pool(name="xin", bufs=2))
    xTrp = ctx.enter_context(tc.tile_pool(name="xTrp", bufs=1))
    ypool = ctx.enter_context(tc.tile_pool(name="ypool", bufs=4))
    zpool = ctx.enter_context(tc.tile_pool(name="zpool", bufs=4))
    wpool = ctx.enter_context(tc.tile_pool(name="wpool", bufs=4))
    Ypool = ctx.enter_context(tc.tile_pool(name="Ypool", bufs=2))
    Opool = ctx.enter_context(tc.tile_pool(name="Opool", bufs=2))
    psT = ctx.enter_context(tc.tile_pool(name="psT", bufs=3, space="PSUM"))
    psM = ctx.enter_context(tc.tile_pool(name="psM", bufs=3, space="PSUM"))

    # constants: round matrices + identity to f32r on device
    amr = consts.tile([128, DIM], f32r, tag="amr")
    bmr = consts.tile([128, DIM], f32r, tag="bmr")
    idt = consts.tile([128, 128], f32, tag="idt")
    idtr = consts.tile([128, 128], f32r, tag="idtr")
    nc.sync.dma_start(idt[:], ident[:])
    nc.vector.tensor_copy(idtr[:], idt[:])
    am_st = mstage.tile([128, DIM], f32, tag="mst")
    for c in range(4):
        nc.sync.dma_start(am_st[:, 1024 * c:1024 * (c + 1)],
                          amats[:, 1024 * c:1024 * (c + 1)])
    for c in range(4):
        eng = nc.vector.tensor_copy if c % 2 else nc.scalar.copy
        eng(amr[:, 1024 * c:1024 * (c + 1)],
            am_st[:, 1024 * c:1024 * (c + 1)])
    bm_st = mstage.tile([128, DIM], f32, tag="mst")
    for c in range(4):
        nc.sync.dma_start(bm_st[:, 1024 * c:1024 * (c + 1)],
                          bmats[:, 1024 * c:1024 * (c + 1)])
    for c in range(4):
        eng = nc.vector.tensor_copy if c % 2 else nc.scalar.copy
        eng(bmr[:, 1024 * c:1024 * (c + 1)],
            bm_st[:, 1024 * c:1024 * (c + 1)])

    load = {"dve": 0.0, "act": 0.0}
    cost = {("dve", "plain"): 685, ("dve", "scatter"): 700,
            ("act", "plain"): 570, ("act", "scatter"): 1127}

    def copy(dst, src, kind="plain"):
        eng = min(("dve", "act"), key=lambda e: load[e] + cost[(e, kind)])
        load[eng] += cost[(eng, kind)]
        (nc.vector.tensor_copy if eng == "dve" else nc.scalar.copy)(dst, src)

    NST = NT // 2            # super-tiles of 256 rows
    for s in range(NST):
        # ---- T1: exact fp32 transposes x -> xTrBig [f', (b, c r-chunk)] f32r
        xTr = xTrp.tile([128, 32 * 256], f32r, tag="xTr")
        for c in range(2):
            xt = xin.tile([128, DIM], f32, tag="xt")
            nc.sync.dma_start(
                xt[:], x[256 * s + 128 * c:256 * s + 128 * (c + 1), :])
            for g in range(8):
                pt = psT.tile([128, 512], f32, tag="psT")
                for j in range(4):
                    b = 4 * g + j
                    nc.tensor.transpose(
                        pt[:, 128 * j:128 * (j + 1)],
                        xt[:, 128 * b:128 * (b + 1)], idt[:])
                # dest: col 256*(4g+j) + 128c + q
                dst = xTr[:].rearrange(
                    "f (bb cc q) -> f cc bb q", bb=32, cc=2, q=128)
                dst = dst[:, c, 4 * g:4 * g + 4]        # [128, 4, 128]
                src = pt[:].rearrange("f (j q) -> f j q", j=4, q=128)
                copy(dst, src)
        # ---- M1 + T2 interleaved per 4-block group: stage A f32r N=256,
        # then f32r transposes y -> Y_c rows-major (b-major contiguous)
        Ys = [Ypool.tile([128, DIM], f32r, tag="Y", name=f"Yc{c}")
              for c in range(2)]
        for g in range(8):
            ySBs = []
            for jj in range(2):
                q = 2 * g + jj
                pm = psM.tile([128, 512], f32, tag="psM")
                for j in range(2):
                    b = 2 * q + j
                    nc.tensor.matmul(
                        pm[:, 256 * j:256 * (j + 1)],
                        amr[:, 128 * b:128 * (b + 1)],
                        xTr[:, 256 * b:256 * (b + 1)],
                        start=True, stop=True)
                ySB = ypool.tile([128, 512], f32r, tag="ySB")
                copy(ySB[:], pm[:])
                ySBs.append(ySB)
            for c in range(2):
                pt = psT.tile([128, 512], f32r, tag="psT")
                for j in range(4):
                    b = 4 * g + j
                    jj, bb = b // 2 - 2 * g, b % 2
                    nc.tensor.transpose(
                        pt[:, 128 * j:128 * (j + 1)],
                        ySBs[jj][:, 256 * bb + 128 * c:256 * bb + 128 * (c + 1)],
                        idtr[:])
                # scatter into f~ order: dest = (p//4)*128 + (p%4)*32 + (4g+j)
                srcv = pt[:].rearrange(
                    "r (j tt pl) -> r j tt pl", j=4, tt=32, pl=4)
                dstv = Ys[c][:].rearrange(
                    "r (tt pl gg j) -> r gg j tt pl",
                    tt=32, pl=4, gg=8, j=4)[:, g]
                copy(dstv, srcv, kind="scatter")
        # ---- T3 + M2 + T4 interleaved per 4-tile group
        Os = [Opool.tile([128, DIM], f32, tag="O", name=f"Oc{c}")
              for c in range(2)]
        for g in range(8):
            wSBs = []
            for jj in range(2):
                q = 2 * g + jj
                pt = psT.tile([128, 512], f32r, tag="psT")
                for j in range(2):
                    t = 2 * q + j
                    for c in range(2):
                        nc.tensor.transpose(
                            pt[:, 256 * j + 128 * c:256 * j + 128 * (c + 1)],
                            Ys[c][:, 128 * t:128 * (t + 1)], idtr[:])
                zr = zpool.tile([128, 512], f32r, tag="zr")
                copy(zr[:], pt[:])
                pw = psM.tile([128, 512], f32, tag="psM")
                for j in range(2):
                    t = 2 * q + j
                    nc.tensor.matmul(
                        pw[:, 256 * j:256 * (j + 1)],
                        bmr[:, 128 * t:128 * (t + 1)],
                        zr[:, 256 * j:256 * (j + 1)],
                        start=True, stop=True)
                wSB = wpool.tile([128, 512], f32r, tag="wSB")
                copy(wSB[:], pw[:])
                wSBs.append(wSB)
            for c in range(2):
                pt = psT.tile([128, 512], f32r, tag="psT")
                for j in range(4):
                    t = 4 * g + j
                    jj, tt = t // 2 - 2 * g, t % 2
                    nc.tensor.transpose(
                        pt[:, 128 * j:128 * (j + 1)],
                        wSBs[jj][:, 256 * tt + 128 * c:256 * tt + 128 * (c + 1)],
                        idtr[:])
                # dest f = b*128 + 16g + 4j + pl ; src col = j*128 + pl*32 + b
                src = pt[:].rearrange("r (j pl b) -> r b j pl", j=4, pl=4, b=32)
                dst = Os[c][:].rearrange(
                    "r (b gg j pl) -> r gg b j pl", b=32, gg=8, j=4, pl=4)[:, g]
                copy(dst, src, kind="scatter")
        for c in range(2):
            nc.sync.dma_start(
                out[256 * s + 128 * c:256 * s + 128 * (c + 1), :], Os[c][:])


RC = 256                 # rows per pipeline chunk (v3)
NCHUNK = RPC // RC       # 4
W3 = 32 * RC             # free width of v3 data tiles


def _emit_kernel_v3(ctx, tc, oT, xT, amats, bmats):
    """bf16 feature-major pipeline, corner turn via SBUF->SBUF DMA.

    Host supplies xT in chunk-major feature-transposed layout:
      xT[c*128 + p, b*RC + r] = x[c*RC + r, 128*b + p]
    Device, per 256-row chunk:
      stage A (weights-stationary): Y^T_b[i, r] = sum_p am[p,128b+i] xT_b[p,r]
        -> Y sbuf [p=f%128 ; (b, r)]
      corner turn: Z[32*pl+bb, RC*t+r] = Y[4*t+pl, RC*bb+r]  (32 plain DMAs)
      stage B: O^T_t[j, r] = sum_q bm[q,128t+j] Z_t[q, r]
        -> oT[c*128 + q', t*RC + r], host un-permutes.
    """
    import concourse.mybir as mybir

    nc = tc.nc
    f32 = mybir.dt.float32
    bf16 = mybir.dt.bfloat16

    consts = ctx.enter_context(tc.tile_pool(name="consts", bufs=1))
    xpool = ctx.enter_context(tc.tile_pool(name="xpool", bufs=3))
    ypool = ctx.enter_context(tc.tile_pool(name="ypool", bufs=2))
    zpool = ctx.enter_context(tc.tile_pool(name="zpool", bufs=2))
    opool = ctx.enter_context(tc.tile_pool(name="opool", bufs=2))
    psA = ctx.enter_context(tc.tile_pool(name="psA", bufs=3, space="PSUM"))
    psB = ctx.enter_context(tc.tile_pool(name="psB", bufs=3, space="PSUM"))

    am = consts.tile([128, DIM], bf16, tag="am")
    bm = consts.tile([128, DIM], bf16, tag="bm")
    for cc in range(4):
        nc.sync.dma_start(am[:, 1024 * cc:1024 * (cc + 1)],
                          amats[:, 1024 * cc:1024 * (cc + 1)])
    for cc in range(4):
        nc.sync.dma_start(bm[:, 1024 * cc:1024 * (cc + 1)],
                          bmats[:, 1024 * cc:1024 * (cc + 1)])

    # greedy DVE/ACT balance for PSUM->SBUF bf16 evacuations of [128, 2*RC]
    load = {"dve": 0.0, "act": 0.0}
    cost = {"dve": 392.0, "act": 357.0}

    def copy(dst, src):
        eng = min(("dve", "act"), key=lambda e: load[e] + cost[e])
        load[eng] += cost[eng]
        (nc.vector.tensor_copy if eng == "dve" else nc.scalar.copy)(dst, src)

    for c in range(NCHUNK):
        xin = xpool.tile([128, W3], bf16, tag="xin")
        nc.sync.dma_start(xin[:], xT[128 * c:128 * (c + 1), :])

        Y = ypool.tile([128, W3], bf16, tag="Y")
        for g in range(16):
            pt = psA.tile([128, 2 * RC], f32, tag="ptA")
            for j in range(2):
                b = 2 * g + j
                nc.tensor.matmul(
                    pt[:, RC * j:RC * (j + 1)],
                    am[:, 128 * b:128 * (b + 1)],
                    xin[:, RC * b:RC * (b + 1)],
                    start=True, stop=True)
            copy(Y[:, 2 * RC * g:2 * RC * (g + 1)], pt[:])

        Z = zpool.tile([128, W3], bf16, tag="Z")
        for t in range(32):
            nc.scalar.dma_start(Z[:, RC * t:RC * (t + 1)], Y[4 * t:4 * t + 4, :])

        O = opool.tile([128, W3], bf16, tag="O")
        for g in range(16):
            pt = psB.tile([128, 2 * RC], f32, tag="ptB")
            for j in range(2):
                t = 2 * g + j
                nc.tensor.matmul(
                    pt[:, RC * j:RC * (j + 1)],
                    bm[:, 128 * t:128 * (t + 1)],
                    Z[:, RC * t:RC * (t + 1)],
                    start=True, stop=True)
            copy(O[:, 2 * RC * g:2 * RC * (g + 1)], pt[:])

        nc.sync.dma_start(oT[128 * c:128 * (c + 1), :], O[:])


def _hoist_matmul_waits(nc):
    """Walrus's fp32/transpose matmul (self-loading LDWEIGHTS) accepts fewer
    sync waits than Tile may assign. Hoist multi-waits onto a PE NoOp inserted
    just before the matmul — same engine queue, so ordering is identical."""
    import concourse.mybir as mybir

    n_hoisted = 0
    for blk in nc.m.functions[0].blocks:
        il = blk.instructions
        i = 0
        while i < len(il):
            inst = il[i]
            si = inst.sync_info
            if (si is not None and len(si.on_wait) > 1
                    and not isinstance(inst, mybir.InstNoOp)):
                waits = list(si.on_wait)
                # keep the last wait on the matmul; one NoOp per extra wait
                # (cayman instructions carry at most one sem-wait each)
                for k, w in enumerate(waits[:-1]):
                    nop = mybir.InstNoOp(
                        name=f"{inst.name}_hw{k}", engine=inst.engine,
                        bass_nofuse=True)
                    nop.sync_info = mybir.SyncInfo(on_wait=[w], on_update=[])
                    nc.register_instruction(nop, overwrite=True)
                    il.insert(i, nop)
                    i += 1
                    n_hoisted += 1
                inst.sync_info = mybir.SyncInfo(
                    on_wait=[waits[-1]], on_update=list(si.on_update))
            i += 1
    return n_hoisted


_CACHED = {}
VARIANT = "v3"   # "v1" fused-fp32 | "v2" f32r | "v3" bf16 + DMA corner turn


def _build_bass(variant=None):
    variant = variant or VARIANT
    if variant in _CACHED:
        return _CACHED[variant]
    from contextlib import ExitStack
    import concourse.bass as bass
    import concourse.tile as tile
    import concourse.mybir as mybir

    f32 = mybir.dt.float32
    bf16 = mybir.dt.bfloat16
    nc = bass.Bass("TRN2", target_bir_lowering=False, debug=False,
                   num_devices=NCORES)
    if variant == "v3":
        xT = nc.dram_tensor("xT", [NCHUNK * 128, W3], bf16,
                            kind="ExternalInput").ap()
        amats = nc.dram_tensor("amats", [128, DIM], bf16,
                               kind="ExternalInput").ap()
        bmats = nc.dram_tensor("bmats", [128, DIM], bf16,
                               kind="ExternalInput").ap()
        oT = nc.dram_tensor("oT", [NCHUNK * 128, W3], bf16,
                            kind="ExternalOutput").ap()
        with tile.TileContext(nc) as tc:
            with ExitStack() as ctx:
                _emit_kernel_v3(ctx, tc, oT, xT, amats, bmats)
    else:
        x = nc.dram_tensor("x", [RPC, DIM], f32, kind="ExternalInput").ap()
        amats = nc.dram_tensor("amats", [128, DIM], f32,
                               kind="ExternalInput").ap()
        bmats = nc.dram_tensor("bmats", [128, DIM], f32,
                               kind="ExternalInput").ap()
        ident = nc.dram_tensor("ident", [128, 128], f32,
                               kind="ExternalInput").ap()
        out = nc.dram_tensor("out", [RPC, DIM], f32, kind="ExternalOutput").ap()

        emit = _emit_kernel if variant == "v1" else _emit_kernel_v2
        with tile.TileContext(nc) as tc:
            with ExitStack() as ctx:
                emit(ctx, tc, out, x, amats, bmats, ident)

    _hoist_matmul_waits(nc)
    _CACHED[variant] = nc
    return nc


def make_in_maps(x, angles):
    x = np.ascontiguousarray(np.asarray(x, np.float32))
    amats, bmats = _build_mats(angles)
    ident = np.eye(128, dtype=np.float32)
    return [
        {"x": x[c * RPC:(c + 1) * RPC], "amats": amats, "bmats": bmats,
         "ident": ident}
        for c in range(NCORES)
    ]


def make_in_maps_v3(x, angles):
    import ml_dtypes
    bf = ml_dtypes.bfloat16
    amats, bmats = _build_mats(angles)
    amb = np.ascontiguousarray(amats.astype(bf))
    bmb = np.ascontiguousarray(bmats.astype(bf))
    x = np.asarray(x, np.float32)
    maps = []
    for c in range(NCORES):
        xc = x[c * RPC:(c + 1) * RPC].astype(bf)        # [RPC, DIM]
        # xT[ch*128 + p, b*RC + r] = xc[ch*RC + r, 128*b + p]
        xp = xc.reshape(NCHUNK, RC, 32, 128).transpose(0, 3, 2, 1)
        xp = np.ascontiguousarray(xp).reshape(NCHUNK * 128, W3)
        maps.append({"xT": xp, "amats": amb, "bmats": bmb})
    return maps


def _unpack_out_v3(oT):
    """oT [NCHUNK*128, W3] bf16 -> [RPC, DIM] f32 in natural order."""
    arr = np.asarray(oT).reshape(NCHUNK, 4, 32, 32, RC)   # [c, pl, b', t, r]
    arr = arr.transpose(0, 4, 2, 3, 1)                    # [c, r, b', t, pl]
    return np.ascontiguousarray(arr).reshape(RPC, DIM).astype(np.float32)


def run_on_hw(x, angles, trace=False, trace_kwargs=None, variant=None):
    from concourse.bass_utils import run_bass_kernel_spmd
    variant = variant or VARIANT
    nc = _build_bass(variant)
    if variant == "v3":
        in_maps = make_in_maps_v3(x, angles)
    else:
        in_maps = make_in_maps(x, angles)
    res = run_bass_kernel_spmd(
        nc, in_maps, core_ids=list(range(NCORES)), trace=trace,
        **(trace_kwargs or {}))
    if variant == "v3":
        out = np.concatenate(
            [_unpack_out_v3(res.results[c]["oT"]) for c in range(NCORES)],
            axis=0)
    else:
        out = np.concatenate(
            [res.results[c]["out"] for c in range(NCORES)], axis=0)
    return out, res


def kernel(x, angles):
    last_err = None
    for attempt in range(3):
        try:
            out, _ = run_on_hw(x, angles, trace=False)
            return np.ascontiguousarray(out.astype(np.float32))
        except Exception as e:  # transient NRT/device errors: retry
            last_err = e
            import time
            time.sleep(5)
    raise last_err



# revision 9
# speedup vs baseline: 1.1292x; 1.1292x over previous
"""Butterfly (Givens) rotation network on TRN2, 8 NeuronCores.

Algorithm
---------
x: (8192, 4096) f32. 12 butterfly layers; layer l rotates pairs of features
differing in bit l of the feature index. Split into two linear stages:

  Stage A = layers 0-6: features mix only within 128-wide blocks b (bits 0-6)
            -> per-block 128x128 matrix A_b.
  Stage B = layers 7-11: features mix only across blocks at fixed within-block
            position p (bits 7-11) -> per-p 32x32 matrix B_p; grouping 4
            consecutive p per 128-partition tile gives block-diag 128x128.

Per 128-row tile (rows on partitions), all on the TensorEngine:
  pass1: per block b: PE-transpose x_b -> xT_b [f',r] (PSUM->SBUF copy),
         MM out[r,fo] = sum_f' xT_b[f',r] * A_bT[f',fo]  (lhsT=xT_b, rhs=A_bT)
         scatter-copy PSUM->SBUF into Y with f~ = (p//4)*128 + (p%4)*32 + b.
  pass2: per f~-tile t: PE-transpose Y_t -> z [f~',r],
         MM out[r,n] = sum z[f~',r] * BDT_t[f~',n], scatter-copy to natural
         feature order, DMA out.

Sharding: data-parallel over rows, 1024 rows/core; matrices replicated.
"""

import math
import numpy as np

DIM = 4096
NL = 12
NB = 32          # 128-wide feature blocks
ROWS = 8192
NCORES = 8
RPC = ROWS // NCORES     # rows per core
NT = RPC // 128          # 128-row tiles per core


# ---------------------------------------------------------------- host math

def _butterfly_np(x, angles):
    """float64 numpy copy of the reference butterfly."""
    x = np.asarray(x, np.float64)
    angles = np.asarray(angles, np.float64)
    B, d = x.shape
    for l in range(angles.shape[0]):
        stride = 2 ** l
        nblocks = d // (2 * stride)
        xr = x.reshape(B, nblocks, 2, stride)
        c = np.cos(angles[l]).reshape(nblocks, stride)
        s = np.sin(angles[l]).reshape(nblocks, stride)
        xi = xr[:, :, 0, :].copy()
        xj = xr[:, :, 1, :].copy()
        x = np.stack([c * xi + s * xj, -s * xi + c * xj], axis=2).reshape(B, d)
    return x


def _build_mats(angles):
    """Returns (amats, bmats) each [128, 4096] f32 in SBUF-ready layout."""
    angles = np.asarray(angles, np.float64)
    amats = np.zeros((128, DIM), np.float64)
    for b in range(NB):
        # A_bT[f_in, f_out]: butterfly of identity rows = F^T for this block
        amats[:, 128 * b:128 * b + 128] = _butterfly_np(
            np.eye(128), angles[0:7, 64 * b:64 * b + 64])
    bmats = np.zeros((128, DIM), np.float64)
    for t in range(32):
        for pl in range(4):
            p = 4 * t + pl
            BpT = _butterfly_np(np.eye(32), angles[7:12, p::128])
            bmats[32 * pl:32 * pl + 32, 128 * t + 32 * pl:128 * t + 32 * pl + 32] = BpT
    return amats.astype(np.float32), bmats.astype(np.float32)


# ---------------------------------------------------------------- bass kernel

def _emit_kernel(ctx, tc, out, x, amats, bmats, ident):
    import concourse.bass as bass
    import concourse.mybir as mybir

    nc = tc.nc
    f32 = mybir.dt.float32

    consts = ctx.enter_context(tc.tile_pool(name="consts", bufs=1))
    xin = ctx.enter_context(tc.tile_pool(name="xin", bufs=3))
    ystage = ctx.enter_context(tc.tile_pool(name="ystage", bufs=3))
    ostage = ctx.enter_context(tc.tile_pool(name="ostage", bufs=3))
    sbst = ctx.enter_context(tc.tile_pool(name="sbst", bufs=6))
    psA = ctx.enter_context(tc.tile_pool(name="psA", bufs=4, space="PSUM"))
    psB = ctx.enter_context(tc.tile_pool(name="psB", bufs=4, space="PSUM"))

    am = consts.tile([128, DIM], f32, tag="amats")
    bm = consts.tile([128, DIM], f32, tag="bmats")
    idt = consts.tile([128, 128], f32, tag="ident")
    nc.sync.dma_start(idt[:], ident[:])

    # Greedy least-loaded assignment of PSUM->SBUF copies to DVE/ACT,
    # using measured per-copy costs (ns) for [128,512] fp32 from PSUM.
    load = {"dve": 0.0, "act": 0.0}
    cost = {("dve", "plain"): 685, ("dve", "scatter"): 700,
            ("act", "plain"): 570, ("act", "scatter"): 1127}

    def copy(dst, src, kind="plain"):
        eng = min(("dve", "act"), key=lambda e: load[e] + cost[(e, kind)])
        load[eng] += cost[(eng, kind)]
        (nc.vector.tensor_copy if eng == "dve" else nc.scalar.copy)(dst, src)

    for i in range(NT):
        xt = xin.tile([128, DIM], f32, tag="xt")
        if i == 0:
            # first tile: fine-grained x/amats chunk interleave so the very
            # first transposes and stage-A matmuls start as early as possible
            for c in range(8):
                nc.sync.dma_start(xt[:, 512 * c:512 * (c + 1)],
                                  x[0:128, 512 * c:512 * (c + 1)])
                nc.sync.dma_start(am[:, 512 * c:512 * (c + 1)],
                                  amats[:, 512 * c:512 * (c + 1)])
        else:
            nc.sync.dma_start(xt[:], x[128 * i:128 * (i + 1), :])
        Y = ystage.tile([128, DIM], f32, tag="Y")

        for g in range(8):           # groups of 4 feature blocks
            pt = psA.tile([128, 512], f32, tag="ptA")
            for j in range(4):
                b = 4 * g + j
                nc.tensor.transpose(
                    pt[:, 128 * j:128 * (j + 1)],
                    xt[:, 128 * b:128 * (b + 1)], idt[:])
            xT4 = sbst.tile([128, 512], f32, tag="xT4")
            copy(xT4[:], pt[:])
            pm = psB.tile([128, 512], f32, tag="pmA")
            for j in range(4):
                b = 4 * g + j
                nc.tensor.matmul(
                    pm[:, 128 * j:128 * (j + 1)],
                    xT4[:, 128 * j:128 * (j + 1)],
                    am[:, 128 * b:128 * (b + 1)],
                    start=True, stop=True)
            # scatter into Y: dest f~ = t*128 + pl*32 + (4g+j), src = j*128 + 4t + pl
            src = pm[:].rearrange("r (j t pl) -> r j t pl", j=4, t=32, pl=4)
            dst = Y[:].rearrange(
                "r (t pl g j) -> r g j t pl", t=32, pl=4, g=8, j=4)[:, g]
            copy(dst, src, kind="scatter")

        if i == 0:
            for c in range(8):
                nc.sync.dma_start(bm[:, 512 * c:512 * (c + 1)],
                                  bmats[:, 512 * c:512 * (c + 1)])
        O = ostage.tile([128, DIM], f32, tag="O")
        for g in range(8):           # groups of 4 f~ tiles
            pt = psA.tile([128, 512], f32, tag="ptA")
            for j in range(4):
                t = 4 * g + j
                nc.tensor.transpose(
                    pt[:, 128 * j:128 * (j + 1)],
                    Y[:, 128 * t:128 * (t + 1)], idt[:])
            z4 = sbst.tile([128, 512], f32, tag="xT4")
            copy(z4[:], pt[:])
            pm = psB.tile([128, 512], f32, tag="pmA")
            for j in range(4):
                t = 4 * g + j
                nc.tensor.matmul(
                    pm[:, 128 * j:128 * (j + 1)],
                    z4[:, 128 * j:128 * (j + 1)],
                    bm[:, 128 * t:128 * (t + 1)],
                    start=True, stop=True)
            # scatter to natural order: dest f = b*128 + 4t + pl = b*128 + 16g + 4j + pl
            src = pm[:].rearrange("r (j pl b) -> r j pl b", j=4, pl=4, b=32)
            dst = O[:].rearrange(
                "r (b g j pl) -> r g j pl b", b=32, g=8, j=4, pl=4)[:, g]
            copy(dst, src, kind="scatter")

        nc.sync.dma_start(out[128 * i:128 * (i + 1), :], O[:])


def _emit_kernel_v2(ctx, tc, out, x, amats, bmats, ident):
    """f32r weights-stationary variant: super-tiles of 256 rows, stage
    matmuls lhsT=matrix rhs=data at N=256 (f32r streams 1 cyc/row vs 4 for
    fp32), data kept feature-major between stages, f32r transposes (1.5
    cyc/row) for all shuffles after the first exact fp32 transpose."""
    import concourse.mybir as mybir

    nc = tc.nc
    f32 = mybir.dt.float32
    f32r = mybir.dt.float32r

    consts = ctx.enter_context(tc.tile_pool(name="consts", bufs=1))
    mstage = ctx.enter_context(tc.tile_pool(name="mstage", bufs=1))
    xin = ctx.enter_context(tc.tile_pool(name="xin", bufs=2))
    xTrp = ctx.enter_context(tc.tile_pool(name="xTrp", bufs=1))
    ypool = ctx.enter_context(tc.tile_pool(name="ypool", bufs=4))
    zpool = ctx.enter_context(tc.tile_pool(name="zpool", bufs=4))
    wpool = ctx.enter_context(tc.tile_pool(name="wpool", bufs=4))
    Ypool = ctx.enter_context(tc.tile_pool(name="Ypool", bufs=2))
    Opool = ctx.enter_context(tc.tile_pool(name="Opool", bufs=2))
    psT = ctx.enter_context(tc.tile_pool(name="psT", bufs=3, space="PSUM"))
    psM = ctx.enter_context(tc.tile_pool(name="psM", bufs=3, space="PSUM"))

    # constants: round matrices + identity to f32r on device
    amr = consts.tile([128, DIM], f32r, tag="amr")
    bmr = consts.tile([128, DIM], f32r, tag="bmr")
    idt = consts.tile([128, 128], f32, tag="idt")
    idtr = consts.tile([128, 128], f32r, tag="idtr")
    nc.sync.dma_start(idt[:], ident[:])
    nc.vector.tensor_copy(idtr[:], idt[:])
    am_st = mstage.tile([128, DIM], f32, tag="mst")
    for c in range(4):
        nc.sync.dma_start(am_st[:, 1024 * c:1024 * (c + 1)],
                          amats[:, 1024 * c:1024 * (c + 1)])
    for c in range(4):
        eng = nc.vector.tensor_copy if c % 2 else nc.scalar.copy
        eng(amr[:, 1024 * c:1024 * (c + 1)],
            am_st[:, 1024 * c:1024 * (c + 1)])
    bm_st = mstage.tile([128, DIM], f32, tag="mst")
    for c in range(4):
        nc.sync.dma_start(bm_st[:, 1024 * c:1024 * (c + 1)],
                          bmats[:, 1024 * c:1024 * (c + 1)])
    for c in range(4):
        eng = nc.vector.tensor_copy if c % 2 else nc.scalar.copy
        eng(bmr[:, 1024 * c:1024 * (c + 1)],
            bm_st[:, 1024 * c:1024 * (c + 1)])

    load = {"dve": 0.0, "act": 0.0}
    cost = {("dve", "plain"): 685, ("dve", "scatter"): 700,
            ("act", "plain"): 570, ("act", "scatter"): 1127}

    def copy(dst, src, kind="plain"):
        eng = min(("dve", "act"), key=lambda e: load[e] + cost[(e, kind)])
        load[eng] += cost[(eng, kind)]
        (nc.vector.tensor_copy if eng == "dve" else nc.scalar.copy)(dst, src)

    NST = NT // 2            # super-tiles of 256 rows
    for s in range(NST):
        # ---- T1: exact fp32 transposes x -> xTrBig [f', (b, c r-chunk)] f32r
        xTr = xTrp.tile([128, 32 * 256], f32r, tag="xTr")
        for c in range(2):
            xt = xin.tile([128, DIM], f32, tag="xt")
            nc.sync.dma_start(
                xt[:], x[256 * s + 128 * c:256 * s + 128 * (c + 1), :])
            for g in range(8):
                pt = psT.tile([128, 512], f32, tag="psT")
                for j in range(4):
                    b = 4 * g + j
                    nc.tensor.transpose(
                        pt[:, 128 * j:128 * (j + 1)],
                        xt[:, 128 * b:128 * (b + 1)], idt[:])
                # dest: col 256*(4g+j) + 128c + q
                dst = xTr[:].rearrange(
                    "f (bb cc q) -> f cc bb q", bb=32, cc=2, q=128)
                dst = dst[:, c, 4 * g:4 * g + 4]        # [128, 4, 128]
                src = pt[:].rearrange("f (j q) -> f j q", j=4, q=128)
                copy(dst, src)
        # ---- M1 + T2 interleaved per 4-block group: stage A f32r N=256,
        # then f32r transposes y -> Y_c rows-major (b-major contiguous)
        Ys = [Ypool.tile([128, DIM], f32r, tag="Y", name=f"Yc{c}")
              for c in range(2)]
        for g in range(8):
            ySBs = []
            for jj in range(2):
                q = 2 * g + jj
                pm = psM.tile([128, 512], f32, tag="psM")
                for j in range(2):
                    b = 2 * q + j
                    nc.tensor.matmul(
                        pm[:, 256 * j:256 * (j + 1)],
                        amr[:, 128 * b:128 * (b + 1)],
                        xTr[:, 256 * b:256 * (b + 1)],
                        start=True, stop=True)
                ySB = ypool.tile([128, 512], f32r, tag="ySB")
                copy(ySB[:], pm[:])
                ySBs.append(ySB)
            for c in range(2):
                pt = psT.tile([128, 512], f32r, tag="psT")
                for j in range(4):
                    b = 4 * g + j
                    jj, bb = b // 2 - 2 * g, b % 2
                    nc.tensor.transpose(
                        pt[:, 128 * j:128 * (j + 1)],
                        ySBs[jj][:, 256 * bb + 128 * c:256 * bb + 128 * (c + 1)],
                        idtr[:])
                # scatter into f~ order: dest = (p//4)*128 + (p%4)*32 + (4g+j)
                srcv = pt[:].rearrange(
                    "r (j tt pl) -> r j tt pl", j=4, tt=32, pl=4)
                dstv = Ys[c][:].rearrange(
                    "r (tt pl gg j) -> r gg j tt pl",
                    tt=32, pl=4, gg=8, j=4)[:, g]
                copy(dstv, srcv, kind="scatter")
        # ---- T3 + M2 + T4 interleaved per 4-tile group
        Os = [Opool.tile([128, DIM], f32, tag="O", name=f"Oc{c}")
              for c in range(2)]
        for g in range(8):
            wSBs = []
            for jj in range(2):
                q = 2 * g + jj
                pt = psT.tile([128, 512], f32r, tag="psT")
                for j in range(2):
                    t = 2 * q + j
                    for c in range(2):
                        nc.tensor.transpose(
                            pt[:, 256 * j + 128 * c:256 * j + 128 * (c + 1)],
                            Ys[c][:, 128 * t:128 * (t + 1)], idtr[:])
                zr = zpool.tile([128, 512], f32r, tag="zr")
                copy(zr[:], pt[:])
                pw = psM.tile([128, 512], f32, tag="psM")
                for j in range(2):
                    t = 2 * q + j
                    nc.tensor.matmul(
                        pw[:, 256 * j:256 * (j + 1)],
                        bmr[:, 128 * t:128 * (t + 1)],
                        zr[:, 256 * j:256 * (j + 1)],
                        start=True, stop=True)
                wSB = wpool.tile([128, 512], f32r, tag="wSB")
                copy(wSB[:], pw[:])
                wSBs.append(wSB)
            for c in range(2):
                pt = psT.tile([128, 512], f32r, tag="psT")
                for j in range(4):
                    t = 4 * g + j
                    jj, tt = t // 2 - 2 * g, t % 2
                    nc.tensor.transpose(
                        pt[:, 128 * j:128 * (j + 1)],
                        wSBs[jj][:, 256 * tt + 128 * c:256 * tt + 128 * (c + 1)],
                        idtr[:])
                # dest f = b*128 + 16g + 4j + pl ; src col = j*128 + pl*32 + b
                src = pt[:].rearrange("r (j pl b) -> r b j pl", j=4, pl=4, b=32)
                dst = Os[c][:].rearrange(
                    "r (b gg j pl) -> r gg b j pl", b=32, gg=8, j=4, pl=4)[:, g]
                copy(dst, src, kind="scatter")
        for c in range(2):
            nc.sync.dma_start(
                out[256 * s + 128 * c:256 * s + 128 * (c + 1), :], Os[c][:])


RC = 256                 # rows per pipeline chunk (v3)
NCHUNK = RPC // RC       # 4
W3 = 32 * RC             # free width of v3 data tiles


def _emit_kernel_v3(ctx, tc, oT, xT, amats, bmats):
    """bf16 feature-major pipeline, corner turn via SBUF->SBUF DMA.

    Host supplies xT in chunk-major feature-transposed layout:
      xT[c*128 + p, b*RC + r] = x[c*RC + r, 128*b + p]
    Device, per 256-row chunk:
      stage A (weights-stationary): Y^T_b[i, r] = sum_p am[p,128b+i] xT_b[p,r]
        -> Y sbuf [p=f%128 ; (b, r)]
      corner turn: Z[32*pl+bb, RC*t+r] = Y[4*t+pl, RC*bb+r]  (32 plain DMAs)
      stage B: O^T_t[j, r] = sum_q bm[q,128t+j] Z_t[q, r]
        -> oT[c*128 + q', t*RC + r], host un-permutes.
    """
    import concourse.mybir as mybir

    nc = tc.nc
    f32 = mybir.dt.float32
    bf16 = mybir.dt.bfloat16

    consts = ctx.enter_context(tc.tile_pool(name="consts", bufs=1))
    xpool = ctx.enter_context(tc.tile_pool(name="xpool", bufs=3))
    ypool = ctx.enter_context(tc.tile_pool(name="ypool", bufs=2))
    zpool = ctx.enter_context(tc.tile_pool(name="zpool", bufs=2))
    opool = ctx.enter_context(tc.tile_pool(name="opool", bufs=2))
    psA = ctx.enter_context(tc.tile_pool(name="psA", bufs=3, space="PSUM"))
    psB = ctx.enter_context(tc.tile_pool(name="psB", bufs=3, space="PSUM"))

    am = consts.tile([128, DIM], bf16, tag="am")
    bm = consts.tile([128, DIM], bf16, tag="bm")
    for cc in range(4):
        nc.sync.dma_start(am[:, 1024 * cc:1024 * (cc + 1)],
                          amats[:, 1024 * cc:1024 * (cc + 1)])
    for cc in range(4):
        nc.sync.dma_start(bm[:, 1024 * cc:1024 * (cc + 1)],
                          bmats[:, 1024 * cc:1024 * (cc + 1)])

    # greedy DVE/ACT balance for PSUM->SBUF bf16 evacuations of [128, 2*RC]
    load = {"dve": 0.0, "act": 0.0}
    cost = {"dve": 392.0, "act": 357.0}

    def copy(dst, src):
        eng = min(("dve", "act"), key=lambda e: load[e] + cost[e])
        load[eng] += cost[eng]
        (nc.vector.tensor_copy if eng == "dve" else nc.scalar.copy)(dst, src)

    for c in range(NCHUNK):
        xin = xpool.tile([128, W3], bf16, tag="xin")
        nc.sync.dma_start(xin[:], xT[128 * c:128 * (c + 1), :])

        Y = ypool.tile([128, W3], bf16, tag="Y")
        for g in range(16):
            pt = psA.tile([128, 2 * RC], f32, tag="ptA")
            for j in range(2):
                b = 2 * g + j
                nc.tensor.matmul(
                    pt[:, RC * j:RC * (j + 1)],
                    am[:, 128 * b:128 * (b + 1)],
                    xin[:, RC * b:RC * (b + 1)],
                    start=True, stop=True)
            copy(Y[:, 2 * RC * g:2 * RC * (g + 1)], pt[:])

        Z = zpool.tile([128, W3], bf16, tag="Z")
        for t in range(32):
            nc.scalar.dma_start(Z[:, RC * t:RC * (t + 1)], Y[4 * t:4 * t + 4, :])

        O = opool.tile([128, W3], bf16, tag="O")
        for g in range(16):
            pt = psB.tile([128, 2 * RC], f32, tag="ptB")
            for j in range(2):
                t = 2 * g + j
                nc.tensor.matmul(
                    pt[:, RC * j:RC * (j + 1)],
                    bm[:, 128 * t:128 * (t + 1)],
                    Z[:, RC * t:RC * (t + 1)],
                    start=True, stop=True)
            copy(O[:, 2 * RC * g:2 * RC * (g + 1)], pt[:])

        nc.sync.dma_start(oT[128 * c:128 * (c + 1), :], O[:])


def _emit_kernel_v4(ctx, tc, oT, xT, amats, bmats):
    """bf16 feature-major pipeline; corner turn on the DVE.

    Stage A's lhsT columns are sigma-permuted on the host so PSUM partition
    s = 32*pl + t holds feature 128*b + 4*t + pl.  The corner turn is then
    quadrant-local -- Z[32*pl+b ; t, r] = Y[32*pl+t ; b, r] -- which is
    exactly the DVE stream-transpose (32x32 blocks) applied per-r via
    strided views: in_ [s; r, b], out [q; r, t].
    """
    import concourse.mybir as mybir

    nc = tc.nc
    f32 = mybir.dt.float32
    bf16 = mybir.dt.bfloat16

    consts = ctx.enter_context(tc.tile_pool(name="consts", bufs=1))
    xpool = ctx.enter_context(tc.tile_pool(name="xpool", bufs=3))
    ypool = ctx.enter_context(tc.tile_pool(name="ypool", bufs=2))
    zpool = ctx.enter_context(tc.tile_pool(name="zpool", bufs=2))
    opool = ctx.enter_context(tc.tile_pool(name="opool", bufs=2))
    psA = ctx.enter_context(tc.tile_pool(name="psA", bufs=2, space="PSUM"))
    psB = ctx.enter_context(tc.tile_pool(name="psB", bufs=2, space="PSUM"))

    am = consts.tile([128, DIM], bf16, tag="am")
    bm = consts.tile([128, DIM], bf16, tag="bm")
    for cc in range(4):
        nc.sync.dma_start(am[:, 1024 * cc:1024 * (cc + 1)],
                          amats[:, 1024 * cc:1024 * (cc + 1)])
    for cc in range(4):
        nc.sync.dma_start(bm[:, 1024 * cc:1024 * (cc + 1)],
                          bmats[:, 1024 * cc:1024 * (cc + 1)])

    # greedy DVE/ACT balance for PSUM->SBUF bf16 evacuations of [128, 1024]
    # (GPSIMD cannot access PSUM -- BIR verifier rejects it)
    load = {"dve": 0.0, "act": 0.0}
    cost = {"dve": 1192.0, "act": 997.0}
    eng_op = {"dve": nc.vector.tensor_copy, "act": nc.scalar.copy}

    def copy(dst, src):
        eng = min(load, key=lambda e: load[e] + cost[e])
        load[eng] += cost[eng]
        eng_op[eng](dst, src)

    NTR = 4                  # corner-turn split (r-slices per chunk)
    RQ = RC // NTR

    for c in range(NCHUNK):
        xin = xpool.tile([128, W3], bf16, tag="xin")
        nc.sync.dma_start(xin[:], xT[128 * c:128 * (c + 1), :])

        Y = ypool.tile([128, W3], bf16, tag="Y")
        for g in range(8):
            pt = psA.tile([128, 1024], f32, tag="ptA")
            for j in range(4):
                b = 4 * g + j
                nc.tensor.matmul(
                    pt[:, RC * j:RC * (j + 1)],
                    am[:, 128 * b:128 * (b + 1)],
                    xin[:, RC * b:RC * (b + 1)],
                    start=True, stop=True)
            copy(Y[:, 1024 * g:1024 * (g + 1)], pt[:])

        Z = zpool.tile([128, W3], bf16, tag="Z")
        yv = Y[:].rearrange("s (b r) -> s b r", b=32, r=RC)
        zv = Z[:].rearrange("q (t r) -> q t r", t=32, r=RC)
        for q in range(NTR):
            inv = yv[:, :, RQ * q:RQ * (q + 1)].transpose([0, 2, 1])
            outv = zv[:, :, RQ * q:RQ * (q + 1)].transpose([0, 2, 1])
            nc.vector.transpose(outv, inv)
            load["dve"] += (58 + 32 * RQ) / 0.96

        O = opool.tile([128, W3], bf16, tag="O")
        for g in range(8):
            pt = psB.tile([128, 1024], f32, tag="ptB")
            for j in range(4):
                t = 4 * g + j
                nc.tensor.matmul(
                    pt[:, RC * j:RC * (j + 1)],
                    bm[:, 128 * t:128 * (t + 1)],
                    Z[:, RC * t:RC * (t + 1)],
                    start=True, stop=True)
            copy(O[:, 1024 * g:1024 * (g + 1)], pt[:])

        nc.sync.dma_start(oT[128 * c:128 * (c + 1), :], O[:])


def _hoist_matmul_waits(nc):
    """Walrus's fp32/transpose matmul (self-loading LDWEIGHTS) accepts fewer
    sync waits than Tile may assign. Hoist multi-waits onto a PE NoOp inserted
    just before the matmul — same engine queue, so ordering is identical."""
    import concourse.mybir as mybir

    n_hoisted = 0
    for blk in nc.m.functions[0].blocks:
        il = blk.instructions
        i = 0
        while i < len(il):
            inst = il[i]
            si = inst.sync_info
            if (si is not None and len(si.on_wait) > 1
                    and not isinstance(inst, mybir.InstNoOp)):
                waits = list(si.on_wait)
                # keep the last wait on the matmul; one NoOp per extra wait
                # (cayman instructions carry at most one sem-wait each)
                for k, w in enumerate(waits[:-1]):
                    nop = mybir.InstNoOp(
                        name=f"{inst.name}_hw{k}", engine=inst.engine,
                        bass_nofuse=True)
                    nop.sync_info = mybir.SyncInfo(on_wait=[w], on_update=[])
                    nc.register_instruction(nop, overwrite=True)
                    il.insert(i, nop)
                    i += 1
                    n_hoisted += 1
                inst.sync_info = mybir.SyncInfo(
                    on_wait=[waits[-1]], on_update=list(si.on_update))
            i += 1
    return n_hoisted


_CACHED = {}
VARIANT = "v4"   # v1 fp32 | v2 f32r | v3 bf16+DMA turn | v4 bf16+DVE turn


def _build_bass(variant=None):
    variant = variant or VARIANT
    if variant in _CACHED:
        return _CACHED[variant]
    from contextlib import ExitStack
    import concourse.bass as bass
    import concourse.tile as tile
    import concourse.mybir as mybir

    f32 = mybir.dt.float32
    bf16 = mybir.dt.bfloat16
    nc = bass.Bass("TRN2", target_bir_lowering=False, debug=False,
                   num_devices=NCORES)
    if variant in ("v3", "v4"):
        xT = nc.dram_tensor("xT", [NCHUNK * 128, W3], bf16,
                            kind="ExternalInput").ap()
        amats = nc.dram_tensor("amats", [128, DIM], bf16,
                               kind="ExternalInput").ap()
        bmats = nc.dram_tensor("bmats", [128, DIM], bf16,
                               kind="ExternalInput").ap()
        oT = nc.dram_tensor("oT", [NCHUNK * 128, W3], bf16,
                            kind="ExternalOutput").ap()
        emit3 = _emit_kernel_v3 if variant == "v3" else _emit_kernel_v4
        with tile.TileContext(nc) as tc:
            with ExitStack() as ctx:
                emit3(ctx, tc, oT, xT, amats, bmats)
    else:
        x = nc.dram_tensor("x", [RPC, DIM], f32, kind="ExternalInput").ap()
        amats = nc.dram_tensor("amats", [128, DIM], f32,
                               kind="ExternalInput").ap()
        bmats = nc.dram_tensor("bmats", [128, DIM], f32,
                               kind="ExternalInput").ap()
        ident = nc.dram_tensor("ident", [128, 128], f32,
                               kind="ExternalInput").ap()
        out = nc.dram_tensor("out", [RPC, DIM], f32, kind="ExternalOutput").ap()

        emit = _emit_kernel if variant == "v1" else _emit_kernel_v2
        with tile.TileContext(nc) as tc:
            with ExitStack() as ctx:
                emit(ctx, tc, out, x, amats, bmats, ident)

    _hoist_matmul_waits(nc)
    _CACHED[variant] = nc
    return nc


def make_in_maps(x, angles):
    x = np.ascontiguousarray(np.asarray(x, np.float32))
    amats, bmats = _build_mats(angles)
    ident = np.eye(128, dtype=np.float32)
    return [
        {"x": x[c * RPC:(c + 1) * RPC], "amats": amats, "bmats": bmats,
         "ident": ident}
        for c in range(NCORES)
    ]


def make_in_maps_v3(x, angles, sigma=False):
    import ml_dtypes
    bf = ml_dtypes.bfloat16
    amats, bmats = _build_mats(angles)
    if sigma:
        # v4: PSUM partition s of block b holds feature 4*(s%32) + s//32
        perm = np.array([4 * (s % 32) + s // 32 for s in range(128)])
        amats = np.ascontiguousarray(
            amats.reshape(128, 32, 128)[:, :, perm].reshape(128, DIM))
    amb = np.ascontiguousarray(amats.astype(bf))
    bmb = np.ascontiguousarray(bmats.astype(bf))
    x = np.asarray(x, np.float32)
    maps = []
    for c in range(NCORES):
        xc = x[c * RPC:(c + 1) * RPC].astype(bf)        # [RPC, DIM]
        # xT[ch*128 + p, b*RC + r] = xc[ch*RC + r, 128*b + p]
        xp = xc.reshape(NCHUNK, RC, 32, 128).transpose(0, 3, 2, 1)
        xp = np.ascontiguousarray(xp).reshape(NCHUNK * 128, W3)
        maps.append({"xT": xp, "amats": amb, "bmats": bmb})
    return maps


def _unpack_out_v3(oT):
    """oT [NCHUNK*128, W3] bf16 -> [RPC, DIM] f32 in natural order."""
    arr = np.asarray(oT).reshape(NCHUNK, 4, 32, 32, RC)   # [c, pl, b', t, r]
    arr = arr.transpose(0, 4, 2, 3, 1)                    # [c, r, b', t, pl]
    return np.ascontiguousarray(arr).reshape(RPC, DIM).astype(np.float32)


def run_on_hw(x, angles, trace=False, trace_kwargs=None, variant=None):
    from concourse.bass_utils import run_bass_kernel_spmd
    variant = variant or VARIANT
    nc = _build_bass(variant)
    if variant in ("v3", "v4"):
        in_maps = make_in_maps_v3(x, angles, sigma=(variant == "v4"))
    else:
        in_maps = make_in_maps(x, angles)
    res = run_bass_kernel_spmd(
        nc, in_maps, core_ids=list(range(NCORES)), trace=trace,
        **(trace_kwargs or {}))
    if variant in ("v3", "v4"):
        out = np.concatenate(
            [_unpack_out_v3(res.results[c]["oT"]) for c in range(NCORES)],
            axis=0)
    else:
        out = np.concatenate(
            [res.results[c]["out"] for c in range(NCORES)], axis=0)
    return out, res


def kernel(x, angles):
    last_err = None
    for attempt in range(3):
        try:
            out, _ = run_on_hw(x, angles, trace=False)
            return np.ascontiguousarray(out.astype(np.float32))
        except Exception as e:  # transient NRT/device errors: retry
            last_err = e
            import time
            time.sleep(5)
    raise last_err



# revision 11
# speedup vs baseline: 1.2868x; 1.1396x over previous
"""Butterfly (Givens) rotation network on TRN2, 8 NeuronCores.

Algorithm
---------
x: (8192, 4096) f32. 12 butterfly layers; layer l rotates pairs of features
differing in bit l of the feature index. Split into two linear stages:

  Stage A = layers 0-6: features mix only within 128-wide blocks b (bits 0-6)
            -> per-block 128x128 matrix A_b.
  Stage B = layers 7-11: features mix only across blocks at fixed within-block
            position p (bits 7-11) -> per-p 32x32 matrix B_p; grouping 4
            consecutive p per 128-partition tile gives block-diag 128x128.

Per 128-row tile (rows on partitions), all on the TensorEngine:
  pass1: per block b: PE-transpose x_b -> xT_b [f',r] (PSUM->SBUF copy),
         MM out[r,fo] = sum_f' xT_b[f',r] * A_bT[f',fo]  (lhsT=xT_b, rhs=A_bT)
         scatter-copy PSUM->SBUF into Y with f~ = (p//4)*128 + (p%4)*32 + b.
  pass2: per f~-tile t: PE-transpose Y_t -> z [f~',r],
         MM out[r,n] = sum z[f~',r] * BDT_t[f~',n], scatter-copy to natural
         feature order, DMA out.

Sharding: data-parallel over rows, 1024 rows/core; matrices replicated.
"""

import math
import numpy as np

DIM = 4096
NL = 12
NB = 32          # 128-wide feature blocks
ROWS = 8192
NCORES = 8
RPC = ROWS // NCORES     # rows per core
NT = RPC // 128          # 128-row tiles per core


# ---------------------------------------------------------------- host math

def _butterfly_np(x, angles):
    """float64 numpy copy of the reference butterfly."""
    x = np.asarray(x, np.float64)
    angles = np.asarray(angles, np.float64)
    B, d = x.shape
    for l in range(angles.shape[0]):
        stride = 2 ** l
        nblocks = d // (2 * stride)
        xr = x.reshape(B, nblocks, 2, stride)
        c = np.cos(angles[l]).reshape(nblocks, stride)
        s = np.sin(angles[l]).reshape(nblocks, stride)
        xi = xr[:, :, 0, :].copy()
        xj = xr[:, :, 1, :].copy()
        x = np.stack([c * xi + s * xj, -s * xi + c * xj], axis=2).reshape(B, d)
    return x


def _build_mats(angles):
    """Returns (amats, bmats) each [128, 4096] f32 in SBUF-ready layout."""
    angles = np.asarray(angles, np.float64)
    amats = np.zeros((128, DIM), np.float64)
    for b in range(NB):
        # A_bT[f_in, f_out]: butterfly of identity rows = F^T for this block
        amats[:, 128 * b:128 * b + 128] = _butterfly_np(
            np.eye(128), angles[0:7, 64 * b:64 * b + 64])
    bmats = np.zeros((128, DIM), np.float64)
    for t in range(32):
        for pl in range(4):
            p = 4 * t + pl
            BpT = _butterfly_np(np.eye(32), angles[7:12, p::128])
            bmats[32 * pl:32 * pl + 32, 128 * t + 32 * pl:128 * t + 32 * pl + 32] = BpT
    return amats.astype(np.float32), bmats.astype(np.float32)


# ---------------------------------------------------------------- bass kernel

def _emit_kernel(ctx, tc, out, x, amats, bmats, ident):
    import concourse.bass as bass
    import concourse.mybir as mybir

    nc = tc.nc
    f32 = mybir.dt.float32

    consts = ctx.enter_context(tc.tile_pool(name="consts", bufs=1))
    xin = ctx.enter_context(tc.tile_pool(name="xin", bufs=3))
    ystage = ctx.enter_context(tc.tile_pool(name="ystage", bufs=3))
    ostage = ctx.enter_context(tc.tile_pool(name="ostage", bufs=3))
    sbst = ctx.enter_context(tc.tile_pool(name="sbst", bufs=6))
    psA = ctx.enter_context(tc.tile_pool(name="psA", bufs=4, space="PSUM"))
    psB = ctx.enter_context(tc.tile_pool(name="psB", bufs=4, space="PSUM"))

    am = consts.tile([128, DIM], f32, tag="amats")
    bm = consts.tile([128, DIM], f32, tag="bmats")
    idt = consts.tile([128, 128], f32, tag="ident")
    nc.sync.dma_start(idt[:], ident[:])

    # Greedy least-loaded assignment of PSUM->SBUF copies to DVE/ACT,
    # using measured per-copy costs (ns) for [128,512] fp32 from PSUM.
    load = {"dve": 0.0, "act": 0.0}
    cost = {("dve", "plain"): 685, ("dve", "scatter"): 700,
            ("act", "plain"): 570, ("act", "scatter"): 1127}

    def copy(dst, src, kind="plain"):
        eng = min(("dve", "act"), key=lambda e: load[e] + cost[(e, kind)])
        load[eng] += cost[(eng, kind)]
        (nc.vector.tensor_copy if eng == "dve" else nc.scalar.copy)(dst, src)

    for i in range(NT):
        xt = xin.tile([128, DIM], f32, tag="xt")
        if i == 0:
            # first tile: fine-grained x/amats chunk interleave so the very
            # first transposes and stage-A matmuls start as early as possible
            for c in range(8):
                nc.sync.dma_start(xt[:, 512 * c:512 * (c + 1)],
                                  x[0:128, 512 * c:512 * (c + 1)])
                nc.sync.dma_start(am[:, 512 * c:512 * (c + 1)],
                                  amats[:, 512 * c:512 * (c + 1)])
        else:
            nc.sync.dma_start(xt[:], x[128 * i:128 * (i + 1), :])
        Y = ystage.tile([128, DIM], f32, tag="Y")

        for g in range(8):           # groups of 4 feature blocks
            pt = psA.tile([128, 512], f32, tag="ptA")
            for j in range(4):
                b = 4 * g + j
                nc.tensor.transpose(
                    pt[:, 128 * j:128 * (j + 1)],
                    xt[:, 128 * b:128 * (b + 1)], idt[:])
            xT4 = sbst.tile([128, 512], f32, tag="xT4")
            copy(xT4[:], pt[:])
            pm = psB.tile([128, 512], f32, tag="pmA")
            for j in range(4):
                b = 4 * g + j
                nc.tensor.matmul(
                    pm[:, 128 * j:128 * (j + 1)],
                    xT4[:, 128 * j:128 * (j + 1)],
                    am[:, 128 * b:128 * (b + 1)],
                    start=True, stop=True)
            # scatter into Y: dest f~ = t*128 + pl*32 + (4g+j), src = j*128 + 4t + pl
            src = pm[:].rearrange("r (j t pl) -> r j t pl", j=4, t=32, pl=4)
            dst = Y[:].rearrange(
                "r (t pl g j) -> r g j t pl", t=32, pl=4, g=8, j=4)[:, g]
            copy(dst, src, kind="scatter")

        if i == 0:
            for c in range(8):
                nc.sync.dma_start(bm[:, 512 * c:512 * (c + 1)],
                                  bmats[:, 512 * c:512 * (c + 1)])
        O = ostage.tile([128, DIM], f32, tag="O")
        for g in range(8):           # groups of 4 f~ tiles
            pt = psA.tile([128, 512], f32, tag="ptA")
            for j in range(4):
                t = 4 * g + j
                nc.tensor.transpose(
                    pt[:, 128 * j:128 * (j + 1)],
                    Y[:, 128 * t:128 * (t + 1)], idt[:])
            z4 = sbst.tile([128, 512], f32, tag="xT4")
            copy(z4[:], pt[:])
            pm = psB.tile([128, 512], f32, tag="pmA")
            for j in range(4):
                t = 4 * g + j
                nc.tensor.matmul(
                    pm[:, 128 * j:128 * (j + 1)],
                    z4[:, 128 * j:128 * (j + 1)],
                    bm[:, 128 * t:128 * (t + 1)],
                    start=True, stop=True)
            # scatter to natural order: dest f = b*128 + 4t + pl = b*128 + 16g + 4j + pl
            src = pm[:].rearrange("r (j pl b) -> r j pl b", j=4, pl=4, b=32)
            dst = O[:].rearrange(
                "r (b g j pl) -> r g j pl b", b=32, g=8, j=4, pl=4)[:, g]
            copy(dst, src, kind="scatter")

        nc.sync.dma_start(out[128 * i:128 * (i + 1), :], O[:])


def _emit_kernel_v2(ctx, tc, out, x, amats, bmats, ident):
    """f32r weights-stationary variant: super-tiles of 256 rows, stage
    matmuls lhsT=matrix rhs=data at N=256 (f32r streams 1 cyc/row vs 4 for
    fp32), data kept feature-major between stages, f32r transposes (1.5
    cyc/row) for all shuffles after the first exact fp32 transpose."""
    import concourse.mybir as mybir

    nc = tc.nc
    f32 = mybir.dt.float32
    f32r = mybir.dt.float32r

    consts = ctx.enter_context(tc.tile_pool(name="consts", bufs=1))
    mstage = ctx.enter_context(tc.tile_pool(name="mstage", bufs=1))
    xin = ctx.enter_context(tc.tile_pool(name="xin", bufs=2))
    xTrp = ctx.enter_context(tc.tile_pool(name="xTrp", bufs=1))
    ypool = ctx.enter_context(tc.tile_pool(name="ypool", bufs=4))
    zpool = ctx.enter_context(tc.tile_pool(name="zpool", bufs=4))
    wpool = ctx.enter_context(tc.tile_pool(name="wpool", bufs=4))
    Ypool = ctx.enter_context(tc.tile_pool(name="Ypool", bufs=2))
    Opool = ctx.enter_context(tc.tile_pool(name="Opool", bufs=2))
    psT = ctx.enter_context(tc.tile_pool(name="psT", bufs=3, space="PSUM"))
    psM = ctx.enter_context(tc.tile_pool(name="psM", bufs=3, space="PSUM"))

    # constants: round matrices + identity to f32r on device
    amr = consts.tile([128, DIM], f32r, tag="amr")
    bmr = consts.tile([128, DIM], f32r, tag="bmr")
    idt = consts.tile([128, 128], f32, tag="idt")
    idtr = consts.tile([128, 128], f32r, tag="idtr")
    nc.sync.dma_start(idt[:], ident[:])
    nc.vector.tensor_copy(idtr[:], idt[:])
    am_st = mstage.tile([128, DIM], f32, tag="mst")
    for c in range(4):
        nc.sync.dma_start(am_st[:, 1024 * c:1024 * (c + 1)],
                          amats[:, 1024 * c:1024 * (c + 1)])
    for c in range(4):
        eng = nc.vector.tensor_copy if c % 2 else nc.scalar.copy
        eng(amr[:, 1024 * c:1024 * (c + 1)],
            am_st[:, 1024 * c:1024 * (c + 1)])
    bm_st = mstage.tile([128, DIM], f32, tag="mst")
    for c in range(4):
        nc.sync.dma_start(bm_st[:, 1024 * c:1024 * (c + 1)],
                          bmats[:, 1024 * c:1024 * (c + 1)])
    for c in range(4):
        eng = nc.vector.tensor_copy if c % 2 else nc.scalar.copy
        eng(bmr[:, 1024 * c:1024 * (c + 1)],
            bm_st[:, 1024 * c:1024 * (c + 1)])

    load = {"dve": 0.0, "act": 0.0}
    cost = {("dve", "plain"): 685, ("dve", "scatter"): 700,
            ("act", "plain"): 570, ("act", "scatter"): 1127}

    def copy(dst, src, kind="plain"):
        eng = min(("dve", "act"), key=lambda e: load[e] + cost[(e, kind)])
        load[eng] += cost[(eng, kind)]
        (nc.vector.tensor_copy if eng == "dve" else nc.scalar.copy)(dst, src)

    NST = NT // 2            # super-tiles of 256 rows
    for s in range(NST):
        # ---- T1: exact fp32 transposes x -> xTrBig [f', (b, c r-chunk)] f32r
        xTr = xTrp.tile([128, 32 * 256], f32r, tag="xTr")
        for c in range(2):
            xt = xin.tile([128, DIM], f32, tag="xt")
            nc.sync.dma_start(
                xt[:], x[256 * s + 128 * c:256 * s + 128 * (c + 1), :])
            for g in range(8):
                pt = psT.tile([128, 512], f32, tag="psT")
                for j in range(4):
                    b = 4 * g + j
                    nc.tensor.transpose(
                        pt[:, 128 * j:128 * (j + 1)],
                        xt[:, 128 * b:128 * (b + 1)], idt[:])
                # dest: col 256*(4g+j) + 128c + q
                dst = xTr[:].rearrange(
                    "f (bb cc q) -> f cc bb q", bb=32, cc=2, q=128)
                dst = dst[:, c, 4 * g:4 * g + 4]        # [128, 4, 128]
                src = pt[:].rearrange("f (j q) -> f j q", j=4, q=128)
                copy(dst, src)
        # ---- M1 + T2 interleaved per 4-block group: stage A f32r N=256,
        # then f32r transposes y -> Y_c rows-major (b-major contiguous)
        Ys = [Ypool.tile([128, DIM], f32r, tag="Y", name=f"Yc{c}")
              for c in range(2)]
        for g in range(8):
            ySBs = []
            for jj in range(2):
                q = 2 * g + jj
                pm = psM.tile([128, 512], f32, tag="psM")
                for j in range(2):
                    b = 2 * q + j
                    nc.tensor.matmul(
                        pm[:, 256 * j:256 * (j + 1)],
                        amr[:, 128 * b:128 * (b + 1)],
                        xTr[:, 256 * b:256 * (b + 1)],
                        start=True, stop=True)
                ySB = ypool.tile([128, 512], f32r, tag="ySB")
                copy(ySB[:], pm[:])
                ySBs.append(ySB)
            for c in range(2):
                pt = psT.tile([128, 512], f32r, tag="psT")
                for j in range(4):
                    b = 4 * g + j
                    jj, bb = b // 2 - 2 * g, b % 2
                    nc.tensor.transpose(
                        pt[:, 128 * j:128 * (j + 1)],
                        ySBs[jj][:, 256 * bb + 128 * c:256 * bb + 128 * (c + 1)],
                        idtr[:])
                # scatter into f~ order: dest = (p//4)*128 + (p%4)*32 + (4g+j)
                srcv = pt[:].rearrange(
                    "r (j tt pl) -> r j tt pl", j=4, tt=32, pl=4)
                dstv = Ys[c][:].rearrange(
                    "r (tt pl gg j) -> r gg j tt pl",
                    tt=32, pl=4, gg=8, j=4)[:, g]
                copy(dstv, srcv, kind="scatter")
        # ---- T3 + M2 + T4 interleaved per 4-tile group
        Os = [Opool.tile([128, DIM], f32, tag="O", name=f"Oc{c}")
              for c in range(2)]
        for g in range(8):
            wSBs = []
            for jj in range(2):
                q = 2 * g + jj
                pt = psT.tile([128, 512], f32r, tag="psT")
                for j in range(2):
                    t = 2 * q + j
                    for c in range(2):
                        nc.tensor.transpose(
                            pt[:, 256 * j + 128 * c:256 * j + 128 * (c + 1)],
                            Ys[c][:, 128 * t:128 * (t + 1)], idtr[:])
                zr = zpool.tile([128, 512], f32r, tag="zr")
                copy(zr[:], pt[:])
                pw = psM.tile([128, 512], f32, tag="psM")
                for j in range(2):
                    t = 2 * q + j
                    nc.tensor.matmul(
                        pw[:, 256 * j:256 * (j + 1)],
                        bmr[:, 128 * t:128 * (t + 1)],
                        zr[:, 256 * j:256 * (j + 1)],
                        start=True, stop=True)
                wSB = wpool.tile([128, 512], f32r, tag="wSB")
                copy(wSB[:], pw[:])
                wSBs.append(wSB)
            for c in range(2):
                pt = psT.tile([128, 512], f32r, tag="psT")
                for j in range(4):
                    t = 4 * g + j
                    jj, tt = t // 2 - 2 * g, t % 2
                    nc.tensor.transpose(
                        pt[:, 128 * j:128 * (j + 1)],
                        wSBs[jj][:, 256 * tt + 128 * c:256 * tt + 128 * (c + 1)],
                        idtr[:])
                # dest f = b*128 + 16g + 4j + pl ; src col = j*128 + pl*32 + b
                src = pt[:].rearrange("r (j pl b) -> r b j pl", j=4, pl=4, b=32)
                dst = Os[c][:].rearrange(
                    "r (b gg j pl) -> r gg b j pl", b=32, gg=8, j=4, pl=4)[:, g]
                copy(dst, src, kind="scatter")
        for c in range(2):
            nc.sync.dma_start(
                out[256 * s + 128 * c:256 * s + 128 * (c + 1), :], Os[c][:])


RC = 256                 # rows per pipeline chunk (v3)
NCHUNK = RPC // RC       # 4
W3 = 32 * RC             # free width of v3 data tiles


def _emit_kernel_v3(ctx, tc, oT, xT, amats, bmats):
    """bf16 feature-major pipeline, corner turn via SBUF->SBUF DMA.

    Host supplies xT in chunk-major feature-transposed layout:
      xT[c*128 + p, b*RC + r] = x[c*RC + r, 128*b + p]
    Device, per 256-row chunk:
      stage A (weights-stationary): Y^T_b[i, r] = sum_p am[p,128b+i] xT_b[p,r]
        -> Y sbuf [p=f%128 ; (b, r)]
      corner turn: Z[32*pl+bb, RC*t+r] = Y[4*t+pl, RC*bb+r]  (32 plain DMAs)
      stage B: O^T_t[j, r] = sum_q bm[q,128t+j] Z_t[q, r]
        -> oT[c*128 + q', t*RC + r], host un-permutes.
    """
    import concourse.mybir as mybir

    nc = tc.nc
    f32 = mybir.dt.float32
    bf16 = mybir.dt.bfloat16

    consts = ctx.enter_context(tc.tile_pool(name="consts", bufs=1))
    xpool = ctx.enter_context(tc.tile_pool(name="xpool", bufs=3))
    ypool = ctx.enter_context(tc.tile_pool(name="ypool", bufs=2))
    zpool = ctx.enter_context(tc.tile_pool(name="zpool", bufs=2))
    opool = ctx.enter_context(tc.tile_pool(name="opool", bufs=2))
    psA = ctx.enter_context(tc.tile_pool(name="psA", bufs=3, space="PSUM"))
    psB = ctx.enter_context(tc.tile_pool(name="psB", bufs=3, space="PSUM"))

    am = consts.tile([128, DIM], bf16, tag="am")
    bm = consts.tile([128, DIM], bf16, tag="bm")
    for cc in range(4):
        nc.sync.dma_start(am[:, 1024 * cc:1024 * (cc + 1)],
                          amats[:, 1024 * cc:1024 * (cc + 1)])
    for cc in range(4):
        nc.sync.dma_start(bm[:, 1024 * cc:1024 * (cc + 1)],
                          bmats[:, 1024 * cc:1024 * (cc + 1)])

    # greedy DVE/ACT balance for PSUM->SBUF bf16 evacuations of [128, 2*RC]
    load = {"dve": 0.0, "act": 0.0}
    cost = {"dve": 392.0, "act": 357.0}

    def copy(dst, src):
        eng = min(("dve", "act"), key=lambda e: load[e] + cost[e])
        load[eng] += cost[eng]
        (nc.vector.tensor_copy if eng == "dve" else nc.scalar.copy)(dst, src)

    for c in range(NCHUNK):
        xin = xpool.tile([128, W3], bf16, tag="xin")
        nc.sync.dma_start(xin[:], xT[128 * c:128 * (c + 1), :])

        Y = ypool.tile([128, W3], bf16, tag="Y")
        for g in range(16):
            pt = psA.tile([128, 2 * RC], f32, tag="ptA")
            for j in range(2):
                b = 2 * g + j
                nc.tensor.matmul(
                    pt[:, RC * j:RC * (j + 1)],
                    am[:, 128 * b:128 * (b + 1)],
                    xin[:, RC * b:RC * (b + 1)],
                    start=True, stop=True)
            copy(Y[:, 2 * RC * g:2 * RC * (g + 1)], pt[:])

        Z = zpool.tile([128, W3], bf16, tag="Z")
        for t in range(32):
            eng = nc.sync if t % 2 == 0 else nc.scalar
            eng.dma_start(Z[:, RC * t:RC * (t + 1)], Y[4 * t:4 * t + 4, :])

        O = opool.tile([128, W3], bf16, tag="O")
        for g in range(16):
            pt = psB.tile([128, 2 * RC], f32, tag="ptB")
            for j in range(2):
                t = 2 * g + j
                nc.tensor.matmul(
                    pt[:, RC * j:RC * (j + 1)],
                    bm[:, 128 * t:128 * (t + 1)],
                    Z[:, RC * t:RC * (t + 1)],
                    start=True, stop=True)
            copy(O[:, 2 * RC * g:2 * RC * (g + 1)], pt[:])

        nc.sync.dma_start(oT[128 * c:128 * (c + 1), :], O[:])


def _emit_kernel_v4(ctx, tc, oT, xT, amats, bmats):
    """bf16 feature-major pipeline; corner turn on the DVE.

    Stage A's lhsT columns are sigma-permuted on the host so PSUM partition
    s = 32*pl + t holds feature 128*b + 4*t + pl.  The corner turn is then
    quadrant-local -- Z[32*pl+b ; t, r] = Y[32*pl+t ; b, r] -- which is
    exactly the DVE stream-transpose (32x32 blocks) applied per-r via
    strided views: in_ [s; r, b], out [q; r, t].
    """
    import concourse.mybir as mybir

    nc = tc.nc
    f32 = mybir.dt.float32
    bf16 = mybir.dt.bfloat16

    consts = ctx.enter_context(tc.tile_pool(name="consts", bufs=1))
    xpool = ctx.enter_context(tc.tile_pool(name="xpool", bufs=3))
    ypool = ctx.enter_context(tc.tile_pool(name="ypool", bufs=2))
    zpool = ctx.enter_context(tc.tile_pool(name="zpool", bufs=2))
    opool = ctx.enter_context(tc.tile_pool(name="opool", bufs=2))
    psA = ctx.enter_context(tc.tile_pool(name="psA", bufs=2, space="PSUM"))
    psB = ctx.enter_context(tc.tile_pool(name="psB", bufs=2, space="PSUM"))

    am = consts.tile([128, DIM], bf16, tag="am")
    bm = consts.tile([128, DIM], bf16, tag="bm")
    for cc in range(4):
        nc.sync.dma_start(am[:, 1024 * cc:1024 * (cc + 1)],
                          amats[:, 1024 * cc:1024 * (cc + 1)])
    for cc in range(4):
        nc.sync.dma_start(bm[:, 1024 * cc:1024 * (cc + 1)],
                          bmats[:, 1024 * cc:1024 * (cc + 1)])

    # greedy DVE/ACT balance for PSUM->SBUF bf16 evacuations of [128, 1024]
    # (GPSIMD cannot access PSUM -- BIR verifier rejects it)
    load = {"dve": 0.0, "act": 0.0}
    cost = {"dve": 1192.0, "act": 997.0}
    eng_op = {"dve": nc.vector.tensor_copy, "act": nc.scalar.copy}

    def copy(dst, src):
        eng = min(load, key=lambda e: load[e] + cost[e])
        load[eng] += cost[eng]
        eng_op[eng](dst, src)

    NTR = 4                  # corner-turn split (r-slices per chunk)
    RQ = RC // NTR

    for c in range(NCHUNK):
        xin = xpool.tile([128, W3], bf16, tag="xin")
        nc.sync.dma_start(xin[:], xT[128 * c:128 * (c + 1), :])

        Y = ypool.tile([128, W3], bf16, tag="Y")
        for g in range(8):
            pt = psA.tile([128, 1024], f32, tag="ptA")
            for j in range(4):
                b = 4 * g + j
                nc.tensor.matmul(
                    pt[:, RC * j:RC * (j + 1)],
                    am[:, 128 * b:128 * (b + 1)],
                    xin[:, RC * b:RC * (b + 1)],
                    start=True, stop=True)
            copy(Y[:, 1024 * g:1024 * (g + 1)], pt[:])

        Z = zpool.tile([128, W3], bf16, tag="Z")
        yv = Y[:].rearrange("s (b r) -> s b r", b=32, r=RC)
        zv = Z[:].rearrange("q (t r) -> q t r", t=32, r=RC)
        for q in range(NTR):
            inv = yv[:, :, RQ * q:RQ * (q + 1)].transpose([0, 2, 1])
            outv = zv[:, :, RQ * q:RQ * (q + 1)].transpose([0, 2, 1])
            nc.vector.transpose(outv, inv)
            load["dve"] += (58 + 32 * RQ) / 0.96

        O = opool.tile([128, W3], bf16, tag="O")
        for g in range(8):
            pt = psB.tile([128, 1024], f32, tag="ptB")
            for j in range(4):
                t = 4 * g + j
                nc.tensor.matmul(
                    pt[:, RC * j:RC * (j + 1)],
                    bm[:, 128 * t:128 * (t + 1)],
                    Z[:, RC * t:RC * (t + 1)],
                    start=True, stop=True)
            copy(O[:, 1024 * g:1024 * (g + 1)], pt[:])

        nc.sync.dma_start(oT[128 * c:128 * (c + 1), :], O[:])


def _hoist_matmul_waits(nc):
    """Walrus's fp32/transpose matmul (self-loading LDWEIGHTS) accepts fewer
    sync waits than Tile may assign. Hoist multi-waits onto a PE NoOp inserted
    just before the matmul — same engine queue, so ordering is identical."""
    import concourse.mybir as mybir

    n_hoisted = 0
    for blk in nc.m.functions[0].blocks:
        il = blk.instructions
        i = 0
        while i < len(il):
            inst = il[i]
            si = inst.sync_info
            if (si is not None and len(si.on_wait) > 1
                    and not isinstance(inst, mybir.InstNoOp)):
                waits = list(si.on_wait)
                # keep the last wait on the matmul; one NoOp per extra wait
                # (cayman instructions carry at most one sem-wait each)
                for k, w in enumerate(waits[:-1]):
                    nop = mybir.InstNoOp(
                        name=f"{inst.name}_hw{k}", engine=inst.engine,
                        bass_nofuse=True)
                    nop.sync_info = mybir.SyncInfo(on_wait=[w], on_update=[])
                    nc.register_instruction(nop, overwrite=True)
                    il.insert(i, nop)
                    i += 1
                    n_hoisted += 1
                inst.sync_info = mybir.SyncInfo(
                    on_wait=[waits[-1]], on_update=list(si.on_update))
            i += 1
    return n_hoisted


_CACHED = {}
VARIANT = "v3"   # v1 fp32 | v2 f32r | v3 bf16+DMA turn | v4 bf16+DVE turn


def _build_bass(variant=None):
    variant = variant or VARIANT
    if variant in _CACHED:
        return _CACHED[variant]
    from contextlib import ExitStack
    import concourse.bass as bass
    import concourse.tile as tile
    import concourse.mybir as mybir

    f32 = mybir.dt.float32
    bf16 = mybir.dt.bfloat16
    nc = bass.Bass("TRN2", target_bir_lowering=False, debug=False,
                   num_devices=NCORES)
    if variant in ("v3", "v4"):
        xT = nc.dram_tensor("xT", [NCHUNK * 128, W3], bf16,
                            kind="ExternalInput").ap()
        amats = nc.dram_tensor("amats", [128, DIM], bf16,
                               kind="ExternalInput").ap()
        bmats = nc.dram_tensor("bmats", [128, DIM], bf16,
                               kind="ExternalInput").ap()
        oT = nc.dram_tensor("oT", [NCHUNK * 128, W3], bf16,
                            kind="ExternalOutput").ap()
        emit3 = _emit_kernel_v3 if variant == "v3" else _emit_kernel_v4
        with tile.TileContext(nc) as tc:
            with ExitStack() as ctx:
                emit3(ctx, tc, oT, xT, amats, bmats)
    else:
        x = nc.dram_tensor("x", [RPC, DIM], f32, kind="ExternalInput").ap()
        amats = nc.dram_tensor("amats", [128, DIM], f32,
                               kind="ExternalInput").ap()
        bmats = nc.dram_tensor("bmats", [128, DIM], f32,
                               kind="ExternalInput").ap()
        ident = nc.dram_tensor("ident", [128, 128], f32,
                               kind="ExternalInput").ap()
        out = nc.dram_tensor("out", [RPC, DIM], f32, kind="ExternalOutput").ap()

        emit = _emit_kernel if variant == "v1" else _emit_kernel_v2
        with tile.TileContext(nc) as tc:
            with ExitStack() as ctx:
                emit(ctx, tc, out, x, amats, bmats, ident)

    _hoist_matmul_waits(nc)
    _CACHED[variant] = nc
    return nc


def make_in_maps(x, angles):
    x = np.ascontiguousarray(np.asarray(x, np.float32))
    amats, bmats = _build_mats(angles)
    ident = np.eye(128, dtype=np.float32)
    return [
        {"x": x[c * RPC:(c + 1) * RPC], "amats": amats, "bmats": bmats,
         "ident": ident}
        for c in range(NCORES)
    ]


def make_in_maps_v3(x, angles, sigma=False):
    import ml_dtypes
    bf = ml_dtypes.bfloat16
    amats, bmats = _build_mats(angles)
    if sigma:
        # v4: PSUM partition s of block b holds feature 4*(s%32) + s//32
        perm = np.array([4 * (s % 32) + s // 32 for s in range(128)])
        amats = np.ascontiguousarray(
            amats.reshape(128, 32, 128)[:, :, perm].reshape(128, DIM))
    amb = np.ascontiguousarray(amats.astype(bf))
    bmb = np.ascontiguousarray(bmats.astype(bf))
    x = np.asarray(x, np.float32)
    maps = []
    for c in range(NCORES):
        xc = x[c * RPC:(c + 1) * RPC].astype(bf)        # [RPC, DIM]
        # xT[ch*128 + p, b*RC + r] = xc[ch*RC + r, 128*b + p]
        xp = xc.reshape(NCHUNK, RC, 32, 128).transpose(0, 3, 2, 1)
        xp = np.ascontiguousarray(xp).reshape(NCHUNK * 128, W3)
        maps.append({"xT": xp, "amats": amb, "bmats": bmb})
    return maps


def _unpack_out_v3(oT):
    """oT [NCHUNK*128, W3] bf16 -> [RPC, DIM] f32 in natural order."""
    arr = np.asarray(oT).reshape(NCHUNK, 4, 32, 32, RC)   # [c, pl, b', t, r]
    arr = arr.transpose(0, 4, 2, 3, 1)                    # [c, r, b', t, pl]
    return np.ascontiguousarray(arr).reshape(RPC, DIM).astype(np.float32)


def run_on_hw(x, angles, trace=False, trace_kwargs=None, variant=None):
    from concourse.bass_utils import run_bass_kernel_spmd
    variant = variant or VARIANT
    nc = _build_bass(variant)
    if variant in ("v3", "v4"):
        in_maps = make_in_maps_v3(x, angles, sigma=(variant == "v4"))
    else:
        in_maps = make_in_maps(x, angles)
    res = run_bass_kernel_spmd(
        nc, in_maps, core_ids=list(range(NCORES)), trace=trace,
        **(trace_kwargs or {}))
    if variant in ("v3", "v4"):
        out = np.concatenate(
            [_unpack_out_v3(res.results[c]["oT"]) for c in range(NCORES)],
            axis=0)
    else:
        out = np.concatenate(
            [res.results[c]["out"] for c in range(NCORES)], axis=0)
    return out, res


def kernel(x, angles):
    last_err = None
    for attempt in range(3):
        try:
            out, _ = run_on_hw(x, angles, trace=False)
            return np.ascontiguousarray(out.astype(np.float32))
        except Exception as e:  # transient NRT/device errors: retry
            last_err = e
            import time
            time.sleep(5)
    raise last_err



# revision 14
# speedup vs baseline: 2.0764x; 1.6137x over previous
"""Butterfly (Givens) rotation network on TRN2, 8 NeuronCores.

Algorithm
---------
x: (8192, 4096) f32. 12 butterfly layers; layer l rotates pairs of features
differing in bit l of the feature index. Split into two linear stages:

  Stage A = layers 0-6: features mix only within 128-wide blocks b (bits 0-6)
            -> per-block 128x128 matrix A_b.
  Stage B = layers 7-11: features mix only across blocks at fixed within-block
            position p (bits 7-11) -> per-p 32x32 matrix B_p; grouping 4
            consecutive p per 128-partition tile gives block-diag 128x128.

Per 128-row tile (rows on partitions), all on the TensorEngine:
  pass1: per block b: PE-transpose x_b -> xT_b [f',r] (PSUM->SBUF copy),
         MM out[r,fo] = sum_f' xT_b[f',r] * A_bT[f',fo]  (lhsT=xT_b, rhs=A_bT)
         scatter-copy PSUM->SBUF into Y with f~ = (p//4)*128 + (p%4)*32 + b.
  pass2: per f~-tile t: PE-transpose Y_t -> z [f~',r],
         MM out[r,n] = sum z[f~',r] * BDT_t[f~',n], scatter-copy to natural
         feature order, DMA out.

Sharding: data-parallel over rows, 1024 rows/core; matrices replicated.
"""

import math
import numpy as np

DIM = 4096
NL = 12
NB = 32          # 128-wide feature blocks
ROWS = 8192
NCORES = 8
RPC = ROWS // NCORES     # rows per core
NT = RPC // 128          # 128-row tiles per core


# ---------------------------------------------------------------- host math

def _butterfly_np(x, angles):
    """float64 numpy copy of the reference butterfly."""
    x = np.asarray(x, np.float64)
    angles = np.asarray(angles, np.float64)
    B, d = x.shape
    for l in range(angles.shape[0]):
        stride = 2 ** l
        nblocks = d // (2 * stride)
        xr = x.reshape(B, nblocks, 2, stride)
        c = np.cos(angles[l]).reshape(nblocks, stride)
        s = np.sin(angles[l]).reshape(nblocks, stride)
        xi = xr[:, :, 0, :].copy()
        xj = xr[:, :, 1, :].copy()
        x = np.stack([c * xi + s * xj, -s * xi + c * xj], axis=2).reshape(B, d)
    return x


def _build_mats(angles):
    """Returns (amats, bmats) each [128, 4096] f32 in SBUF-ready layout."""
    angles = np.asarray(angles, np.float64)
    amats = np.zeros((128, DIM), np.float64)
    for b in range(NB):
        # A_bT[f_in, f_out]: butterfly of identity rows = F^T for this block
        amats[:, 128 * b:128 * b + 128] = _butterfly_np(
            np.eye(128), angles[0:7, 64 * b:64 * b + 64])
    bmats = np.zeros((128, DIM), np.float64)
    for t in range(32):
        for pl in range(4):
            p = 4 * t + pl
            BpT = _butterfly_np(np.eye(32), angles[7:12, p::128])
            bmats[32 * pl:32 * pl + 32, 128 * t + 32 * pl:128 * t + 32 * pl + 32] = BpT
    return amats.astype(np.float32), bmats.astype(np.float32)


# ---------------------------------------------------------------- bass kernel

def _emit_kernel(ctx, tc, out, x, amats, bmats, ident):
    import concourse.bass as bass
    import concourse.mybir as mybir

    nc = tc.nc
    f32 = mybir.dt.float32

    consts = ctx.enter_context(tc.tile_pool(name="consts", bufs=1))
    xin = ctx.enter_context(tc.tile_pool(name="xin", bufs=3))
    ystage = ctx.enter_context(tc.tile_pool(name="ystage", bufs=3))
    ostage = ctx.enter_context(tc.tile_pool(name="ostage", bufs=3))
    sbst = ctx.enter_context(tc.tile_pool(name="sbst", bufs=6))
    psA = ctx.enter_context(tc.tile_pool(name="psA", bufs=4, space="PSUM"))
    psB = ctx.enter_context(tc.tile_pool(name="psB", bufs=4, space="PSUM"))

    am = consts.tile([128, DIM], f32, tag="amats")
    bm = consts.tile([128, DIM], f32, tag="bmats")
    idt = consts.tile([128, 128], f32, tag="ident")
    nc.sync.dma_start(idt[:], ident[:])

    # Greedy least-loaded assignment of PSUM->SBUF copies to DVE/ACT,
    # using measured per-copy costs (ns) for [128,512] fp32 from PSUM.
    load = {"dve": 0.0, "act": 0.0}
    cost = {("dve", "plain"): 685, ("dve", "scatter"): 700,
            ("act", "plain"): 570, ("act", "scatter"): 1127}

    def copy(dst, src, kind="plain"):
        eng = min(("dve", "act"), key=lambda e: load[e] + cost[(e, kind)])
        load[eng] += cost[(eng, kind)]
        (nc.vector.tensor_copy if eng == "dve" else nc.scalar.copy)(dst, src)

    for i in range(NT):
        xt = xin.tile([128, DIM], f32, tag="xt")
        if i == 0:
            # first tile: fine-grained x/amats chunk interleave so the very
            # first transposes and stage-A matmuls start as early as possible
            for c in range(8):
                nc.sync.dma_start(xt[:, 512 * c:512 * (c + 1)],
                                  x[0:128, 512 * c:512 * (c + 1)])
                nc.sync.dma_start(am[:, 512 * c:512 * (c + 1)],
                                  amats[:, 512 * c:512 * (c + 1)])
        else:
            nc.sync.dma_start(xt[:], x[128 * i:128 * (i + 1), :])
        Y = ystage.tile([128, DIM], f32, tag="Y")

        for g in range(8):           # groups of 4 feature blocks
            pt = psA.tile([128, 512], f32, tag="ptA")
            for j in range(4):
                b = 4 * g + j
                nc.tensor.transpose(
                    pt[:, 128 * j:128 * (j + 1)],
                    xt[:, 128 * b:128 * (b + 1)], idt[:])
            xT4 = sbst.tile([128, 512], f32, tag="xT4")
            copy(xT4[:], pt[:])
            pm = psB.tile([128, 512], f32, tag="pmA")
            for j in range(4):
                b = 4 * g + j
                nc.tensor.matmul(
                    pm[:, 128 * j:128 * (j + 1)],
                    xT4[:, 128 * j:128 * (j + 1)],
                    am[:, 128 * b:128 * (b + 1)],
                    start=True, stop=True)
            # scatter into Y: dest f~ = t*128 + pl*32 + (4g+j), src = j*128 + 4t + pl
            src = pm[:].rearrange("r (j t pl) -> r j t pl", j=4, t=32, pl=4)
            dst = Y[:].rearrange(
                "r (t pl g j) -> r g j t pl", t=32, pl=4, g=8, j=4)[:, g]
            copy(dst, src, kind="scatter")

        if i == 0:
            for c in range(8):
                nc.sync.dma_start(bm[:, 512 * c:512 * (c + 1)],
                                  bmats[:, 512 * c:512 * (c + 1)])
        O = ostage.tile([128, DIM], f32, tag="O")
        for g in range(8):           # groups of 4 f~ tiles
            pt = psA.tile([128, 512], f32, tag="ptA")
            for j in range(4):
                t = 4 * g + j
                nc.tensor.transpose(
                    pt[:, 128 * j:128 * (j + 1)],
                    Y[:, 128 * t:128 * (t + 1)], idt[:])
            z4 = sbst.tile([128, 512], f32, tag="xT4")
            copy(z4[:], pt[:])
            pm = psB.tile([128, 512], f32, tag="pmA")
            for j in range(4):
                t = 4 * g + j
                nc.tensor.matmul(
                    pm[:, 128 * j:128 * (j + 1)],
                    z4[:, 128 * j:128 * (j + 1)],
                    bm[:, 128 * t:128 * (t + 1)],
                    start=True, stop=True)
            # scatter to natural order: dest f = b*128 + 4t + pl = b*128 + 16g + 4j + pl
            src = pm[:].rearrange("r (j pl b) -> r j pl b", j=4, pl=4, b=32)
            dst = O[:].rearrange(
                "r (b g j pl) -> r g j pl b", b=32, g=8, j=4, pl=4)[:, g]
            copy(dst, src, kind="scatter")

        nc.sync.dma_start(out[128 * i:128 * (i + 1), :], O[:])


def _emit_kernel_v2(ctx, tc, out, x, amats, bmats, ident):
    """f32r weights-stationary variant: super-tiles of 256 rows, stage
    matmuls lhsT=matrix rhs=data at N=256 (f32r streams 1 cyc/row vs 4 for
    fp32), data kept feature-major between stages, f32r transposes (1.5
    cyc/row) for all shuffles after the first exact fp32 transpose."""
    import concourse.mybir as mybir

    nc = tc.nc
    f32 = mybir.dt.float32
    f32r = mybir.dt.float32r

    consts = ctx.enter_context(tc.tile_pool(name="consts", bufs=1))
    mstage = ctx.enter_context(tc.tile_pool(name="mstage", bufs=1))
    xin = ctx.enter_context(tc.tile_pool(name="xin", bufs=2))
    xTrp = ctx.enter_context(tc.tile_pool(name="xTrp", bufs=1))
    ypool = ctx.enter_context(tc.tile_pool(name="ypool", bufs=4))
    zpool = ctx.enter_context(tc.tile_pool(name="zpool", bufs=4))
    wpool = ctx.enter_context(tc.tile_pool(name="wpool", bufs=4))
    Ypool = ctx.enter_context(tc.tile_pool(name="Ypool", bufs=2))
    Opool = ctx.enter_context(tc.tile_pool(name="Opool", bufs=2))
    psT = ctx.enter_context(tc.tile_pool(name="psT", bufs=3, space="PSUM"))
    psM = ctx.enter_context(tc.tile_pool(name="psM", bufs=3, space="PSUM"))

    # constants: round matrices + identity to f32r on device
    amr = consts.tile([128, DIM], f32r, tag="amr")
    bmr = consts.tile([128, DIM], f32r, tag="bmr")
    idt = consts.tile([128, 128], f32, tag="idt")
    idtr = consts.tile([128, 128], f32r, tag="idtr")
    nc.sync.dma_start(idt[:], ident[:])
    nc.vector.tensor_copy(idtr[:], idt[:])
    am_st = mstage.tile([128, DIM], f32, tag="mst")
    for c in range(4):
        nc.sync.dma_start(am_st[:, 1024 * c:1024 * (c + 1)],
                          amats[:, 1024 * c:1024 * (c + 1)])
    for c in range(4):
        eng = nc.vector.tensor_copy if c % 2 else nc.scalar.copy
        eng(amr[:, 1024 * c:1024 * (c + 1)],
            am_st[:, 1024 * c:1024 * (c + 1)])
    bm_st = mstage.tile([128, DIM], f32, tag="mst")
    for c in range(4):
        nc.sync.dma_start(bm_st[:, 1024 * c:1024 * (c + 1)],
                          bmats[:, 1024 * c:1024 * (c + 1)])
    for c in range(4):
        eng = nc.vector.tensor_copy if c % 2 else nc.scalar.copy
        eng(bmr[:, 1024 * c:1024 * (c + 1)],
            bm_st[:, 1024 * c:1024 * (c + 1)])

    load = {"dve": 0.0, "act": 0.0}
    cost = {("dve", "plain"): 685, ("dve", "scatter"): 700,
            ("act", "plain"): 570, ("act", "scatter"): 1127}

    def copy(dst, src, kind="plain"):
        eng = min(("dve", "act"), key=lambda e: load[e] + cost[(e, kind)])
        load[eng] += cost[(eng, kind)]
        (nc.vector.tensor_copy if eng == "dve" else nc.scalar.copy)(dst, src)

    NST = NT // 2            # super-tiles of 256 rows
    for s in range(NST):
        # ---- T1: exact fp32 transposes x -> xTrBig [f', (b, c r-chunk)] f32r
        xTr = xTrp.tile([128, 32 * 256], f32r, tag="xTr")
        for c in range(2):
            xt = xin.tile([128, DIM], f32, tag="xt")
            nc.sync.dma_start(
                xt[:], x[256 * s + 128 * c:256 * s + 128 * (c + 1), :])
            for g in range(8):
                pt = psT.tile([128, 512], f32, tag="psT")
                for j in range(4):
                    b = 4 * g + j
                    nc.tensor.transpose(
                        pt[:, 128 * j:128 * (j + 1)],
                        xt[:, 128 * b:128 * (b + 1)], idt[:])
                # dest: col 256*(4g+j) + 128c + q
                dst = xTr[:].rearrange(
                    "f (bb cc q) -> f cc bb q", bb=32, cc=2, q=128)
                dst = dst[:, c, 4 * g:4 * g + 4]        # [128, 4, 128]
                src = pt[:].rearrange("f (j q) -> f j q", j=4, q=128)
                copy(dst, src)
        # ---- M1 + T2 interleaved per 4-block group: stage A f32r N=256,
        # then f32r transposes y -> Y_c rows-major (b-major contiguous)
        Ys = [Ypool.tile([128, DIM], f32r, tag="Y", name=f"Yc{c}")
              for c in range(2)]
        for g in range(8):
            ySBs = []
            for jj in range(2):
                q = 2 * g + jj
                pm = psM.tile([128, 512], f32, tag="psM")
                for j in range(2):
                    b = 2 * q + j
                    nc.tensor.matmul(
                        pm[:, 256 * j:256 * (j + 1)],
                        amr[:, 128 * b:128 * (b + 1)],
                        xTr[:, 256 * b:256 * (b + 1)],
                        start=True, stop=True)
                ySB = ypool.tile([128, 512], f32r, tag="ySB")
                copy(ySB[:], pm[:])
                ySBs.append(ySB)
            for c in range(2):
                pt = psT.tile([128, 512], f32r, tag="psT")
                for j in range(4):
                    b = 4 * g + j
                    jj, bb = b // 2 - 2 * g, b % 2
                    nc.tensor.transpose(
                        pt[:, 128 * j:128 * (j + 1)],
                        ySBs[jj][:, 256 * bb + 128 * c:256 * bb + 128 * (c + 1)],
                        idtr[:])
                # scatter into f~ order: dest = (p//4)*128 + (p%4)*32 + (4g+j)
                srcv = pt[:].rearrange(
                    "r (j tt pl) -> r j tt pl", j=4, tt=32, pl=4)
                dstv = Ys[c][:].rearrange(
                    "r (tt pl gg j) -> r gg j tt pl",
                    tt=32, pl=4, gg=8, j=4)[:, g]
                copy(dstv, srcv, kind="scatter")
        # ---- T3 + M2 + T4 interleaved per 4-tile group
        Os = [Opool.tile([128, DIM], f32, tag="O", name=f"Oc{c}")
              for c in range(2)]
        for g in range(8):
            wSBs = []
            for jj in range(2):
                q = 2 * g + jj
                pt = psT.tile([128, 512], f32r, tag="psT")
                for j in range(2):
                    t = 2 * q + j
                    for c in range(2):
                        nc.tensor.transpose(
                            pt[:, 256 * j + 128 * c:256 * j + 128 * (c + 1)],
                            Ys[c][:, 128 * t:128 * (t + 1)], idtr[:])
                zr = zpool.tile([128, 512], f32r, tag="zr")
                copy(zr[:], pt[:])
                pw = psM.tile([128, 512], f32, tag="psM")
                for j in range(2):
                    t = 2 * q + j
                    nc.tensor.matmul(
                        pw[:, 256 * j:256 * (j + 1)],
                        bmr[:, 128 * t:128 * (t + 1)],
                        zr[:, 256 * j:256 * (j + 1)],
                        start=True, stop=True)
                wSB = wpool.tile([128, 512], f32r, tag="wSB")
                copy(wSB[:], pw[:])
                wSBs.append(wSB)
            for c in range(2):
                pt = psT.tile([128, 512], f32r, tag="psT")
                for j in range(4):
                    t = 4 * g + j
                    jj, tt = t // 2 - 2 * g, t % 2
                    nc.tensor.transpose(
                        pt[:, 128 * j:128 * (j + 1)],
                        wSBs[jj][:, 256 * tt + 128 * c:256 * tt + 128 * (c + 1)],
                        idtr[:])
                # dest f = b*128 + 16g + 4j + pl ; src col = j*128 + pl*32 + b
                src = pt[:].rearrange("r (j pl b) -> r b j pl", j=4, pl=4, b=32)
                dst = Os[c][:].rearrange(
                    "r (b gg j pl) -> r gg b j pl", b=32, gg=8, j=4, pl=4)[:, g]
                copy(dst, src, kind="scatter")
        for c in range(2):
            nc.sync.dma_start(
                out[256 * s + 128 * c:256 * s + 128 * (c + 1), :], Os[c][:])


RC = 256                 # rows per pipeline chunk (v3)
NCHUNK = RPC // RC       # 4
W3 = 32 * RC             # free width of v3 data tiles


def _emit_kernel_v3(ctx, tc, oT, xT, amats, bmats):
    """bf16 feature-major pipeline, corner turn via SBUF->SBUF DMA.

    Host supplies xT in chunk-major feature-transposed layout:
      xT[c*128 + p, b*RC + r] = x[c*RC + r, 128*b + p]
    Device, per 256-row chunk:
      stage A (weights-stationary): Y^T_b[i, r] = sum_p am[p,128b+i] xT_b[p,r]
        -> Y sbuf [p=f%128 ; (b, r)]
      corner turn: Z[32*pl+bb, RC*t+r] = Y[4*t+pl, RC*bb+r]  (32 plain DMAs)
      stage B: O^T_t[j, r] = sum_q bm[q,128t+j] Z_t[q, r]
        -> oT[c*128 + q', t*RC + r], host un-permutes.
    """
    import concourse.mybir as mybir

    nc = tc.nc
    f32 = mybir.dt.float32
    bf16 = mybir.dt.bfloat16

    consts = ctx.enter_context(tc.tile_pool(name="consts", bufs=1))
    xpool = ctx.enter_context(tc.tile_pool(name="xpool", bufs=3))
    ypool = ctx.enter_context(tc.tile_pool(name="ypool", bufs=2))
    zpool = ctx.enter_context(tc.tile_pool(name="zpool", bufs=2))
    opool = ctx.enter_context(tc.tile_pool(name="opool", bufs=2))
    psA = ctx.enter_context(tc.tile_pool(name="psA", bufs=3, space="PSUM"))
    psB = ctx.enter_context(tc.tile_pool(name="psB", bufs=3, space="PSUM"))

    am = consts.tile([128, DIM], bf16, tag="am")
    bm = consts.tile([128, DIM], bf16, tag="bm")
    for cc in range(4):
        nc.sync.dma_start(am[:, 1024 * cc:1024 * (cc + 1)],
                          amats[:, 1024 * cc:1024 * (cc + 1)])
    for cc in range(4):
        nc.sync.dma_start(bm[:, 1024 * cc:1024 * (cc + 1)],
                          bmats[:, 1024 * cc:1024 * (cc + 1)])

    # greedy DVE/ACT balance for PSUM->SBUF bf16 evacuations of [128, 2*RC]
    load = {"dve": 0.0, "act": 0.0}
    cost = {"dve": 392.0, "act": 357.0}

    def copy(dst, src):
        eng = min(("dve", "act"), key=lambda e: load[e] + cost[e])
        load[eng] += cost[eng]
        (nc.vector.tensor_copy if eng == "dve" else nc.scalar.copy)(dst, src)

    for c in range(NCHUNK):
        xin = xpool.tile([128, W3], bf16, tag="xin")
        nc.sync.dma_start(xin[:], xT[128 * c:128 * (c + 1), :])

        Y = ypool.tile([128, W3], bf16, tag="Y")
        for g in range(16):
            pt = psA.tile([128, 2 * RC], f32, tag="ptA")
            for j in range(2):
                b = 2 * g + j
                nc.tensor.matmul(
                    pt[:, RC * j:RC * (j + 1)],
                    am[:, 128 * b:128 * (b + 1)],
                    xin[:, RC * b:RC * (b + 1)],
                    start=True, stop=True)
            copy(Y[:, 2 * RC * g:2 * RC * (g + 1)], pt[:])

        Z = zpool.tile([128, W3], bf16, tag="Z")
        for t in range(32):
            eng = nc.sync if t % 2 == 0 else nc.scalar
            eng.dma_start(Z[:, RC * t:RC * (t + 1)], Y[4 * t:4 * t + 4, :])

        O = opool.tile([128, W3], bf16, tag="O")
        for g in range(16):
            pt = psB.tile([128, 2 * RC], f32, tag="ptB")
            for j in range(2):
                t = 2 * g + j
                nc.tensor.matmul(
                    pt[:, RC * j:RC * (j + 1)],
                    bm[:, 128 * t:128 * (t + 1)],
                    Z[:, RC * t:RC * (t + 1)],
                    start=True, stop=True)
            copy(O[:, 2 * RC * g:2 * RC * (g + 1)], pt[:])

        nc.sync.dma_start(oT[128 * c:128 * (c + 1), :], O[:])


def _emit_kernel_v4(ctx, tc, oT, xT, amats, bmats):
    """bf16 feature-major pipeline; corner turn on the DVE.

    Stage A's lhsT columns are sigma-permuted on the host so PSUM partition
    s = 32*pl + t holds feature 128*b + 4*t + pl.  The corner turn is then
    quadrant-local -- Z[32*pl+b ; t, r] = Y[32*pl+t ; b, r] -- which is
    exactly the DVE stream-transpose (32x32 blocks) applied per-r via
    strided views: in_ [s; r, b], out [q; r, t].
    """
    import concourse.mybir as mybir

    nc = tc.nc
    f32 = mybir.dt.float32
    bf16 = mybir.dt.bfloat16

    consts = ctx.enter_context(tc.tile_pool(name="consts", bufs=1))
    xpool = ctx.enter_context(tc.tile_pool(name="xpool", bufs=3))
    ypool = ctx.enter_context(tc.tile_pool(name="ypool", bufs=2))
    zpool = ctx.enter_context(tc.tile_pool(name="zpool", bufs=2))
    opool = ctx.enter_context(tc.tile_pool(name="opool", bufs=2))
    psA = ctx.enter_context(tc.tile_pool(name="psA", bufs=2, space="PSUM"))
    psB = ctx.enter_context(tc.tile_pool(name="psB", bufs=2, space="PSUM"))

    am = consts.tile([128, DIM], bf16, tag="am")
    bm = consts.tile([128, DIM], bf16, tag="bm")
    for cc in range(4):
        nc.sync.dma_start(am[:, 1024 * cc:1024 * (cc + 1)],
                          amats[:, 1024 * cc:1024 * (cc + 1)])
    for cc in range(4):
        nc.sync.dma_start(bm[:, 1024 * cc:1024 * (cc + 1)],
                          bmats[:, 1024 * cc:1024 * (cc + 1)])

    # greedy DVE/ACT balance for PSUM->SBUF bf16 evacuations of [128, 1024]
    # (GPSIMD cannot access PSUM -- BIR verifier rejects it)
    load = {"dve": 0.0, "act": 0.0}
    cost = {"dve": 1192.0, "act": 997.0}
    eng_op = {"dve": nc.vector.tensor_copy, "act": nc.scalar.copy}

    def copy(dst, src):
        eng = min(load, key=lambda e: load[e] + cost[e])
        load[eng] += cost[eng]
        eng_op[eng](dst, src)

    NTR = 4                  # corner-turn split (r-slices per chunk)
    RQ = RC // NTR

    for c in range(NCHUNK):
        xin = xpool.tile([128, W3], bf16, tag="xin")
        nc.sync.dma_start(xin[:], xT[128 * c:128 * (c + 1), :])

        Y = ypool.tile([128, W3], bf16, tag="Y")
        for g in range(8):
            pt = psA.tile([128, 1024], f32, tag="ptA")
            for j in range(4):
                b = 4 * g + j
                nc.tensor.matmul(
                    pt[:, RC * j:RC * (j + 1)],
                    am[:, 128 * b:128 * (b + 1)],
                    xin[:, RC * b:RC * (b + 1)],
                    start=True, stop=True)
            copy(Y[:, 1024 * g:1024 * (g + 1)], pt[:])

        Z = zpool.tile([128, W3], bf16, tag="Z")
        yv = Y[:].rearrange("s (b r) -> s b r", b=32, r=RC)
        zv = Z[:].rearrange("q (t r) -> q t r", t=32, r=RC)
        for q in range(NTR):
            inv = yv[:, :, RQ * q:RQ * (q + 1)].transpose([0, 2, 1])
            outv = zv[:, :, RQ * q:RQ * (q + 1)].transpose([0, 2, 1])
            nc.vector.transpose(outv, inv)
            load["dve"] += (58 + 32 * RQ) / 0.96

        O = opool.tile([128, W3], bf16, tag="O")
        for g in range(8):
            pt = psB.tile([128, 1024], f32, tag="ptB")
            for j in range(4):
                t = 4 * g + j
                nc.tensor.matmul(
                    pt[:, RC * j:RC * (j + 1)],
                    bm[:, 128 * t:128 * (t + 1)],
                    Z[:, RC * t:RC * (t + 1)],
                    start=True, stop=True)
            copy(O[:, 1024 * g:1024 * (g + 1)], pt[:])

        nc.sync.dma_start(oT[128 * c:128 * (c + 1), :], O[:])


def _emit_kernel_v5(ctx, tc, oT, xT, amats, bmats, ydram):
    """bf16 feature-major pipeline; corner turn via HBM round trip.

    The f~ relabeling (f~ = 32*s + b = 128*t + 32*pl + b, s = 4*t + pl) is
    exactly the row-major flattening of Y's (s, b) indices, so the Y->DRAM
    write per chunk is one PLAIN contiguous 2 MB transfer (full line rate,
    16-engine spread).  The read-back gathers Z[q; t, r] = ydram[128t+q, r]
    with a clean 3-dim AP (runs of RC elements, dst = 128 partitions).
    """
    import concourse.mybir as mybir

    nc = tc.nc
    f32 = mybir.dt.float32
    bf16 = mybir.dt.bfloat16

    consts = ctx.enter_context(tc.tile_pool(name="consts", bufs=1))
    xpool = ctx.enter_context(tc.tile_pool(name="xpool", bufs=3))
    ypool = ctx.enter_context(tc.tile_pool(name="ypool", bufs=2))
    zpool = ctx.enter_context(tc.tile_pool(name="zpool", bufs=2))
    opool = ctx.enter_context(tc.tile_pool(name="opool", bufs=2))
    psA = ctx.enter_context(tc.tile_pool(name="psA", bufs=2, space="PSUM"))
    psB = ctx.enter_context(tc.tile_pool(name="psB", bufs=2, space="PSUM"))

    am = consts.tile([128, DIM], bf16, tag="am")
    bm = consts.tile([128, DIM], bf16, tag="bm")
    for cc in range(4):
        nc.sync.dma_start(am[:, 1024 * cc:1024 * (cc + 1)],
                          amats[:, 1024 * cc:1024 * (cc + 1)])
    for cc in range(4):
        nc.sync.dma_start(bm[:, 1024 * cc:1024 * (cc + 1)],
                          bmats[:, 1024 * cc:1024 * (cc + 1)])

    load = {"dve": 0.0, "act": 0.0}
    cost = {"dve": 1192.0, "act": 997.0}
    eng_op = {"dve": nc.vector.tensor_copy, "act": nc.scalar.copy}

    def copy(dst, src):
        eng = min(load, key=lambda e: load[e] + cost[e])
        load[eng] += cost[eng]
        eng_op[eng](dst, src)

    # ydram: [NCHUNK * 4096, RC] bf16; chunk c rows [4096c, 4096(c+1))
    for c in range(NCHUNK):
        xin = xpool.tile([128, W3], bf16, tag="xin")
        nc.sync.dma_start(xin[:], xT[128 * c:128 * (c + 1), :])

        Y = ypool.tile([128, W3], bf16, tag="Y")
        for g in range(8):
            pt = psA.tile([128, 1024], f32, tag="ptA")
            for j in range(4):
                b = 4 * g + j
                nc.tensor.matmul(
                    pt[:, RC * j:RC * (j + 1)],
                    am[:, 128 * b:128 * (b + 1)],
                    xin[:, RC * b:RC * (b + 1)],
                    start=True, stop=True)
            copy(Y[:, 1024 * g:1024 * (g + 1)], pt[:])

        # corner turn, hop 1: flat contiguous write (f~ = 32 s + b)
        ywr = ydram[4096 * c:4096 * (c + 1), :].rearrange(
            "(s b) r -> s b r", s=128, b=32)
        nc.sync.dma_start(ywr, Y[:].rearrange("s (b r) -> s b r", b=32))

        # hop 2: Z[q; t, r] = ydram[4096 c + 128 t + q, r]
        Z = zpool.tile([128, W3], bf16, tag="Z")
        zrd = ydram[4096 * c:4096 * (c + 1), :].rearrange(
            "(t q) r -> q t r", t=32, q=128)
        nc.scalar.dma_start(Z[:].rearrange("q (t r) -> q t r", t=32), zrd)

        O = opool.tile([128, W3], bf16, tag="O")
        for g in range(8):
            pt = psB.tile([128, 1024], f32, tag="ptB")
            for j in range(4):
                t = 4 * g + j
                nc.tensor.matmul(
                    pt[:, RC * j:RC * (j + 1)],
                    bm[:, 128 * t:128 * (t + 1)],
                    Z[:, RC * t:RC * (t + 1)],
                    start=True, stop=True)
            copy(O[:, 1024 * g:1024 * (g + 1)], pt[:])

        nc.sync.dma_start(oT[128 * c:128 * (c + 1), :], O[:])


def _hoist_matmul_waits(nc):
    """Walrus's fp32/transpose matmul (self-loading LDWEIGHTS) accepts fewer
    sync waits than Tile may assign. Hoist multi-waits onto a PE NoOp inserted
    just before the matmul — same engine queue, so ordering is identical."""
    import concourse.mybir as mybir

    n_hoisted = 0
    for blk in nc.m.functions[0].blocks:
        il = blk.instructions
        i = 0
        while i < len(il):
            inst = il[i]
            si = inst.sync_info
            if (si is not None and len(si.on_wait) > 1
                    and not isinstance(inst, mybir.InstNoOp)):
                waits = list(si.on_wait)
                # keep the last wait on the matmul; one NoOp per extra wait
                # (cayman instructions carry at most one sem-wait each)
                for k, w in enumerate(waits[:-1]):
                    nop = mybir.InstNoOp(
                        name=f"{inst.name}_hw{k}", engine=inst.engine,
                        bass_nofuse=True)
                    nop.sync_info = mybir.SyncInfo(on_wait=[w], on_update=[])
                    nc.register_instruction(nop, overwrite=True)
                    il.insert(i, nop)
                    i += 1
                    n_hoisted += 1
                inst.sync_info = mybir.SyncInfo(
                    on_wait=[waits[-1]], on_update=list(si.on_update))
            i += 1
    return n_hoisted


_CACHED = {}
VARIANT = "v5"   # v1 fp32 | v2 f32r | v3 bf16+DMA turn | v4 bf16+DVE turn


def _build_bass(variant=None):
    variant = variant or VARIANT
    if variant in _CACHED:
        return _CACHED[variant]
    from contextlib import ExitStack
    import concourse.bass as bass
    import concourse.tile as tile
    import concourse.mybir as mybir

    f32 = mybir.dt.float32
    bf16 = mybir.dt.bfloat16
    nc = bass.Bass("TRN2", target_bir_lowering=False, debug=False,
                   num_devices=NCORES)
    if variant in ("v3", "v4", "v5"):
        xT = nc.dram_tensor("xT", [NCHUNK * 128, W3], bf16,
                            kind="ExternalInput").ap()
        amats = nc.dram_tensor("amats", [128, DIM], bf16,
                               kind="ExternalInput").ap()
        bmats = nc.dram_tensor("bmats", [128, DIM], bf16,
                               kind="ExternalInput").ap()
        oT = nc.dram_tensor("oT", [NCHUNK * 128, W3], bf16,
                            kind="ExternalOutput").ap()
        if variant == "v5":
            ydram = nc.dram_tensor("ydram", [NCHUNK * DIM, RC], bf16,
                                   kind="Internal").ap()
            with tile.TileContext(nc) as tc:
                with ExitStack() as ctx:
                    _emit_kernel_v5(ctx, tc, oT, xT, amats, bmats, ydram)
        else:
            emit3 = _emit_kernel_v3 if variant == "v3" else _emit_kernel_v4
            with tile.TileContext(nc) as tc:
                with ExitStack() as ctx:
                    emit3(ctx, tc, oT, xT, amats, bmats)
    else:
        x = nc.dram_tensor("x", [RPC, DIM], f32, kind="ExternalInput").ap()
        amats = nc.dram_tensor("amats", [128, DIM], f32,
                               kind="ExternalInput").ap()
        bmats = nc.dram_tensor("bmats", [128, DIM], f32,
                               kind="ExternalInput").ap()
        ident = nc.dram_tensor("ident", [128, 128], f32,
                               kind="ExternalInput").ap()
        out = nc.dram_tensor("out", [RPC, DIM], f32, kind="ExternalOutput").ap()

        emit = _emit_kernel if variant == "v1" else _emit_kernel_v2
        with tile.TileContext(nc) as tc:
            with ExitStack() as ctx:
                emit(ctx, tc, out, x, amats, bmats, ident)

    _hoist_matmul_waits(nc)
    _CACHED[variant] = nc
    return nc


def make_in_maps(x, angles):
    x = np.ascontiguousarray(np.asarray(x, np.float32))
    amats, bmats = _build_mats(angles)
    ident = np.eye(128, dtype=np.float32)
    return [
        {"x": x[c * RPC:(c + 1) * RPC], "amats": amats, "bmats": bmats,
         "ident": ident}
        for c in range(NCORES)
    ]


def make_in_maps_v3(x, angles, sigma=False):
    import ml_dtypes
    bf = ml_dtypes.bfloat16
    amats, bmats = _build_mats(angles)
    if sigma:
        # v4: PSUM partition s of block b holds feature 4*(s%32) + s//32
        perm = np.array([4 * (s % 32) + s // 32 for s in range(128)])
        amats = np.ascontiguousarray(
            amats.reshape(128, 32, 128)[:, :, perm].reshape(128, DIM))
    amb = np.ascontiguousarray(amats.astype(bf))
    bmb = np.ascontiguousarray(bmats.astype(bf))
    x = np.asarray(x, np.float32)
    maps = []
    for c in range(NCORES):
        xc = x[c * RPC:(c + 1) * RPC].astype(bf)        # [RPC, DIM]
        # xT[ch*128 + p, b*RC + r] = xc[ch*RC + r, 128*b + p]
        xp = xc.reshape(NCHUNK, RC, 32, 128).transpose(0, 3, 2, 1)
        xp = np.ascontiguousarray(xp).reshape(NCHUNK * 128, W3)
        maps.append({"xT": xp, "amats": amb, "bmats": bmb})
    return maps


def _unpack_out_v3(oT):
    """oT [NCHUNK*128, W3] bf16 -> [RPC, DIM] f32 in natural order."""
    arr = np.asarray(oT).reshape(NCHUNK, 4, 32, 32, RC)   # [c, pl, b', t, r]
    arr = arr.transpose(0, 4, 2, 3, 1)                    # [c, r, b', t, pl]
    return np.ascontiguousarray(arr).reshape(RPC, DIM).astype(np.float32)


def run_on_hw(x, angles, trace=False, trace_kwargs=None, variant=None):
    from concourse.bass_utils import run_bass_kernel_spmd
    variant = variant or VARIANT
    nc = _build_bass(variant)
    if variant in ("v3", "v4", "v5"):
        in_maps = make_in_maps_v3(x, angles, sigma=(variant == "v4"))
    else:
        in_maps = make_in_maps(x, angles)
    res = run_bass_kernel_spmd(
        nc, in_maps, core_ids=list(range(NCORES)), trace=trace,
        **(trace_kwargs or {}))
    if variant in ("v3", "v4", "v5"):
        out = np.concatenate(
            [_unpack_out_v3(res.results[c]["oT"]) for c in range(NCORES)],
            axis=0)
    else:
        out = np.concatenate(
            [res.results[c]["out"] for c in range(NCORES)], axis=0)
    return out, res


def kernel(x, angles):
    last_err = None
    for attempt in range(3):
        try:
            out, _ = run_on_hw(x, angles, trace=False)
            return np.ascontiguousarray(out.astype(np.float32))
        except Exception as e:  # transient NRT/device errors: retry
            last_err = e
            import time
            time.sleep(5)
    raise last_err



# revision 16
# speedup vs baseline: 2.4007x; 1.1562x over previous
"""Butterfly (Givens) rotation network on TRN2, 8 NeuronCores.

Algorithm
---------
x: (8192, 4096) f32. 12 butterfly layers; layer l rotates pairs of features
differing in bit l of the feature index. Split into two linear stages:

  Stage A = layers 0-6: features mix only within 128-wide blocks b (bits 0-6)
            -> per-block 128x128 matrix A_b.
  Stage B = layers 7-11: features mix only across blocks at fixed within-block
            position p (bits 7-11) -> per-p 32x32 matrix B_p; grouping 4
            consecutive p per 128-partition tile gives block-diag 128x128.

Per 128-row tile (rows on partitions), all on the TensorEngine:
  pass1: per block b: PE-transpose x_b -> xT_b [f',r] (PSUM->SBUF copy),
         MM out[r,fo] = sum_f' xT_b[f',r] * A_bT[f',fo]  (lhsT=xT_b, rhs=A_bT)
         scatter-copy PSUM->SBUF into Y with f~ = (p//4)*128 + (p%4)*32 + b.
  pass2: per f~-tile t: PE-transpose Y_t -> z [f~',r],
         MM out[r,n] = sum z[f~',r] * BDT_t[f~',n], scatter-copy to natural
         feature order, DMA out.

Sharding: data-parallel over rows, 1024 rows/core; matrices replicated.
"""

import math
import numpy as np

DIM = 4096
NL = 12
NB = 32          # 128-wide feature blocks
ROWS = 8192
NCORES = 8
RPC = ROWS // NCORES     # rows per core
NT = RPC // 128          # 128-row tiles per core


# ---------------------------------------------------------------- host math

def _butterfly_np(x, angles):
    """float64 numpy copy of the reference butterfly."""
    x = np.asarray(x, np.float64)
    angles = np.asarray(angles, np.float64)
    B, d = x.shape
    for l in range(angles.shape[0]):
        stride = 2 ** l
        nblocks = d // (2 * stride)
        xr = x.reshape(B, nblocks, 2, stride)
        c = np.cos(angles[l]).reshape(nblocks, stride)
        s = np.sin(angles[l]).reshape(nblocks, stride)
        xi = xr[:, :, 0, :].copy()
        xj = xr[:, :, 1, :].copy()
        x = np.stack([c * xi + s * xj, -s * xi + c * xj], axis=2).reshape(B, d)
    return x


def _build_mats(angles):
    """Returns (amats, bmats) each [128, 4096] f32 in SBUF-ready layout."""
    angles = np.asarray(angles, np.float64)
    amats = np.zeros((128, DIM), np.float64)
    for b in range(NB):
        # A_bT[f_in, f_out]: butterfly of identity rows = F^T for this block
        amats[:, 128 * b:128 * b + 128] = _butterfly_np(
            np.eye(128), angles[0:7, 64 * b:64 * b + 64])
    bmats = np.zeros((128, DIM), np.float64)
    for t in range(32):
        for pl in range(4):
            p = 4 * t + pl
            BpT = _butterfly_np(np.eye(32), angles[7:12, p::128])
            bmats[32 * pl:32 * pl + 32, 128 * t + 32 * pl:128 * t + 32 * pl + 32] = BpT
    return amats.astype(np.float32), bmats.astype(np.float32)


# ---------------------------------------------------------------- bass kernel

def _emit_kernel(ctx, tc, out, x, amats, bmats, ident):
    import concourse.bass as bass
    import concourse.mybir as mybir

    nc = tc.nc
    f32 = mybir.dt.float32

    consts = ctx.enter_context(tc.tile_pool(name="consts", bufs=1))
    xin = ctx.enter_context(tc.tile_pool(name="xin", bufs=3))
    ystage = ctx.enter_context(tc.tile_pool(name="ystage", bufs=3))
    ostage = ctx.enter_context(tc.tile_pool(name="ostage", bufs=3))
    sbst = ctx.enter_context(tc.tile_pool(name="sbst", bufs=6))
    psA = ctx.enter_context(tc.tile_pool(name="psA", bufs=4, space="PSUM"))
    psB = ctx.enter_context(tc.tile_pool(name="psB", bufs=4, space="PSUM"))

    am = consts.tile([128, DIM], f32, tag="amats")
    bm = consts.tile([128, DIM], f32, tag="bmats")
    idt = consts.tile([128, 128], f32, tag="ident")
    nc.sync.dma_start(idt[:], ident[:])

    # Greedy least-loaded assignment of PSUM->SBUF copies to DVE/ACT,
    # using measured per-copy costs (ns) for [128,512] fp32 from PSUM.
    load = {"dve": 0.0, "act": 0.0}
    cost = {("dve", "plain"): 685, ("dve", "scatter"): 700,
            ("act", "plain"): 570, ("act", "scatter"): 1127}

    def copy(dst, src, kind="plain"):
        eng = min(("dve", "act"), key=lambda e: load[e] + cost[(e, kind)])
        load[eng] += cost[(eng, kind)]
        (nc.vector.tensor_copy if eng == "dve" else nc.scalar.copy)(dst, src)

    for i in range(NT):
        xt = xin.tile([128, DIM], f32, tag="xt")
        if i == 0:
            # first tile: fine-grained x/amats chunk interleave so the very
            # first transposes and stage-A matmuls start as early as possible
            for c in range(8):
                nc.sync.dma_start(xt[:, 512 * c:512 * (c + 1)],
                                  x[0:128, 512 * c:512 * (c + 1)])
                nc.sync.dma_start(am[:, 512 * c:512 * (c + 1)],
                                  amats[:, 512 * c:512 * (c + 1)])
        else:
            nc.sync.dma_start(xt[:], x[128 * i:128 * (i + 1), :])
        Y = ystage.tile([128, DIM], f32, tag="Y")

        for g in range(8):           # groups of 4 feature blocks
            pt = psA.tile([128, 512], f32, tag="ptA")
            for j in range(4):
                b = 4 * g + j
                nc.tensor.transpose(
                    pt[:, 128 * j:128 * (j + 1)],
                    xt[:, 128 * b:128 * (b + 1)], idt[:])
            xT4 = sbst.tile([128, 512], f32, tag="xT4")
            copy(xT4[:], pt[:])
            pm = psB.tile([128, 512], f32, tag="pmA")
            for j in range(4):
                b = 4 * g + j
                nc.tensor.matmul(
                    pm[:, 128 * j:128 * (j + 1)],
                    xT4[:, 128 * j:128 * (j + 1)],
                    am[:, 128 * b:128 * (b + 1)],
                    start=True, stop=True)
            # scatter into Y: dest f~ = t*128 + pl*32 + (4g+j), src = j*128 + 4t + pl
            src = pm[:].rearrange("r (j t pl) -> r j t pl", j=4, t=32, pl=4)
            dst = Y[:].rearrange(
                "r (t pl g j) -> r g j t pl", t=32, pl=4, g=8, j=4)[:, g]
            copy(dst, src, kind="scatter")

        if i == 0:
            for c in range(8):
                nc.sync.dma_start(bm[:, 512 * c:512 * (c + 1)],
                                  bmats[:, 512 * c:512 * (c + 1)])
        O = ostage.tile([128, DIM], f32, tag="O")
        for g in range(8):           # groups of 4 f~ tiles
            pt = psA.tile([128, 512], f32, tag="ptA")
            for j in range(4):
                t = 4 * g + j
                nc.tensor.transpose(
                    pt[:, 128 * j:128 * (j + 1)],
                    Y[:, 128 * t:128 * (t + 1)], idt[:])
            z4 = sbst.tile([128, 512], f32, tag="xT4")
            copy(z4[:], pt[:])
            pm = psB.tile([128, 512], f32, tag="pmA")
            for j in range(4):
                t = 4 * g + j
                nc.tensor.matmul(
                    pm[:, 128 * j:128 * (j + 1)],
                    z4[:, 128 * j:128 * (j + 1)],
                    bm[:, 128 * t:128 * (t + 1)],
                    start=True, stop=True)
            # scatter to natural order: dest f = b*128 + 4t + pl = b*128 + 16g + 4j + pl
            src = pm[:].rearrange("r (j pl b) -> r j pl b", j=4, pl=4, b=32)
            dst = O[:].rearrange(
                "r (b g j pl) -> r g j pl b", b=32, g=8, j=4, pl=4)[:, g]
            copy(dst, src, kind="scatter")

        nc.sync.dma_start(out[128 * i:128 * (i + 1), :], O[:])


def _emit_kernel_v2(ctx, tc, out, x, amats, bmats, ident):
    """f32r weights-stationary variant: super-tiles of 256 rows, stage
    matmuls lhsT=matrix rhs=data at N=256 (f32r streams 1 cyc/row vs 4 for
    fp32), data kept feature-major between stages, f32r transposes (1.5
    cyc/row) for all shuffles after the first exact fp32 transpose."""
    import concourse.mybir as mybir

    nc = tc.nc
    f32 = mybir.dt.float32
    f32r = mybir.dt.float32r

    consts = ctx.enter_context(tc.tile_pool(name="consts", bufs=1))
    mstage = ctx.enter_context(tc.tile_pool(name="mstage", bufs=1))
    xin = ctx.enter_context(tc.tile_pool(name="xin", bufs=2))
    xTrp = ctx.enter_context(tc.tile_pool(name="xTrp", bufs=1))
    ypool = ctx.enter_context(tc.tile_pool(name="ypool", bufs=4))
    zpool = ctx.enter_context(tc.tile_pool(name="zpool", bufs=4))
    wpool = ctx.enter_context(tc.tile_pool(name="wpool", bufs=4))
    Ypool = ctx.enter_context(tc.tile_pool(name="Ypool", bufs=2))
    Opool = ctx.enter_context(tc.tile_pool(name="Opool", bufs=2))
    psT = ctx.enter_context(tc.tile_pool(name="psT", bufs=3, space="PSUM"))
    psM = ctx.enter_context(tc.tile_pool(name="psM", bufs=3, space="PSUM"))

    # constants: round matrices + identity to f32r on device
    amr = consts.tile([128, DIM], f32r, tag="amr")
    bmr = consts.tile([128, DIM], f32r, tag="bmr")
    idt = consts.tile([128, 128], f32, tag="idt")
    idtr = consts.tile([128, 128], f32r, tag="idtr")
    nc.sync.dma_start(idt[:], ident[:])
    nc.vector.tensor_copy(idtr[:], idt[:])
    am_st = mstage.tile([128, DIM], f32, tag="mst")
    for c in range(4):
        nc.sync.dma_start(am_st[:, 1024 * c:1024 * (c + 1)],
                          amats[:, 1024 * c:1024 * (c + 1)])
    for c in range(4):
        eng = nc.vector.tensor_copy if c % 2 else nc.scalar.copy
        eng(amr[:, 1024 * c:1024 * (c + 1)],
            am_st[:, 1024 * c:1024 * (c + 1)])
    bm_st = mstage.tile([128, DIM], f32, tag="mst")
    for c in range(4):
        nc.sync.dma_start(bm_st[:, 1024 * c:1024 * (c + 1)],
                          bmats[:, 1024 * c:1024 * (c + 1)])
    for c in range(4):
        eng = nc.vector.tensor_copy if c % 2 else nc.scalar.copy
        eng(bmr[:, 1024 * c:1024 * (c + 1)],
            bm_st[:, 1024 * c:1024 * (c + 1)])

    load = {"dve": 0.0, "act": 0.0}
    cost = {("dve", "plain"): 685, ("dve", "scatter"): 700,
            ("act", "plain"): 570, ("act", "scatter"): 1127}

    def copy(dst, src, kind="plain"):
        eng = min(("dve", "act"), key=lambda e: load[e] + cost[(e, kind)])
        load[eng] += cost[(eng, kind)]
        (nc.vector.tensor_copy if eng == "dve" else nc.scalar.copy)(dst, src)

    NST = NT // 2            # super-tiles of 256 rows
    for s in range(NST):
        # ---- T1: exact fp32 transposes x -> xTrBig [f', (b, c r-chunk)] f32r
        xTr = xTrp.tile([128, 32 * 256], f32r, tag="xTr")
        for c in range(2):
            xt = xin.tile([128, DIM], f32, tag="xt")
            nc.sync.dma_start(
                xt[:], x[256 * s + 128 * c:256 * s + 128 * (c + 1), :])
            for g in range(8):
                pt = psT.tile([128, 512], f32, tag="psT")
                for j in range(4):
                    b = 4 * g + j
                    nc.tensor.transpose(
                        pt[:, 128 * j:128 * (j + 1)],
                        xt[:, 128 * b:128 * (b + 1)], idt[:])
                # dest: col 256*(4g+j) + 128c + q
                dst = xTr[:].rearrange(
                    "f (bb cc q) -> f cc bb q", bb=32, cc=2, q=128)
                dst = dst[:, c, 4 * g:4 * g + 4]        # [128, 4, 128]
                src = pt[:].rearrange("f (j q) -> f j q", j=4, q=128)
                copy(dst, src)
        # ---- M1 + T2 interleaved per 4-block group: stage A f32r N=256,
        # then f32r transposes y -> Y_c rows-major (b-major contiguous)
        Ys = [Ypool.tile([128, DIM], f32r, tag="Y", name=f"Yc{c}")
              for c in range(2)]
        for g in range(8):
            ySBs = []
            for jj in range(2):
                q = 2 * g + jj
                pm = psM.tile([128, 512], f32, tag="psM")
                for j in range(2):
                    b = 2 * q + j
                    nc.tensor.matmul(
                        pm[:, 256 * j:256 * (j + 1)],
                        amr[:, 128 * b:128 * (b + 1)],
                        xTr[:, 256 * b:256 * (b + 1)],
                        start=True, stop=True)
                ySB = ypool.tile([128, 512], f32r, tag="ySB")
                copy(ySB[:], pm[:])
                ySBs.append(ySB)
            for c in range(2):
                pt = psT.tile([128, 512], f32r, tag="psT")
                for j in range(4):
                    b = 4 * g + j
                    jj, bb = b // 2 - 2 * g, b % 2
                    nc.tensor.transpose(
                        pt[:, 128 * j:128 * (j + 1)],
                        ySBs[jj][:, 256 * bb + 128 * c:256 * bb + 128 * (c + 1)],
                        idtr[:])
                # scatter into f~ order: dest = (p//4)*128 + (p%4)*32 + (4g+j)
                srcv = pt[:].rearrange(
                    "r (j tt pl) -> r j tt pl", j=4, tt=32, pl=4)
                dstv = Ys[c][:].rearrange(
                    "r (tt pl gg j) -> r gg j tt pl",
                    tt=32, pl=4, gg=8, j=4)[:, g]
                copy(dstv, srcv, kind="scatter")
        # ---- T3 + M2 + T4 interleaved per 4-tile group
        Os = [Opool.tile([128, DIM], f32, tag="O", name=f"Oc{c}")
              for c in range(2)]
        for g in range(8):
            wSBs = []
            for jj in range(2):
                q = 2 * g + jj
                pt = psT.tile([128, 512], f32r, tag="psT")
                for j in range(2):
                    t = 2 * q + j
                    for c in range(2):
                        nc.tensor.transpose(
                            pt[:, 256 * j + 128 * c:256 * j + 128 * (c + 1)],
                            Ys[c][:, 128 * t:128 * (t + 1)], idtr[:])
                zr = zpool.tile([128, 512], f32r, tag="zr")
                copy(zr[:], pt[:])
                pw = psM.tile([128, 512], f32, tag="psM")
                for j in range(2):
                    t = 2 * q + j
                    nc.tensor.matmul(
                        pw[:, 256 * j:256 * (j + 1)],
                        bmr[:, 128 * t:128 * (t + 1)],
                        zr[:, 256 * j:256 * (j + 1)],
                        start=True, stop=True)
                wSB = wpool.tile([128, 512], f32r, tag="wSB")
                copy(wSB[:], pw[:])
                wSBs.append(wSB)
            for c in range(2):
                pt = psT.tile([128, 512], f32r, tag="psT")
                for j in range(4):
                    t = 4 * g + j
                    jj, tt = t // 2 - 2 * g, t % 2
                    nc.tensor.transpose(
                        pt[:, 128 * j:128 * (j + 1)],
                        wSBs[jj][:, 256 * tt + 128 * c:256 * tt + 128 * (c + 1)],
                        idtr[:])
                # dest f = b*128 + 16g + 4j + pl ; src col = j*128 + pl*32 + b
                src = pt[:].rearrange("r (j pl b) -> r b j pl", j=4, pl=4, b=32)
                dst = Os[c][:].rearrange(
                    "r (b gg j pl) -> r gg b j pl", b=32, gg=8, j=4, pl=4)[:, g]
                copy(dst, src, kind="scatter")
        for c in range(2):
            nc.sync.dma_start(
                out[256 * s + 128 * c:256 * s + 128 * (c + 1), :], Os[c][:])


RC = 256                 # rows per pipeline chunk (v3)
NCHUNK = RPC // RC       # 4
W3 = 32 * RC             # free width of v3 data tiles


def _emit_kernel_v3(ctx, tc, oT, xT, amats, bmats):
    """bf16 feature-major pipeline, corner turn via SBUF->SBUF DMA.

    Host supplies xT in chunk-major feature-transposed layout:
      xT[c*128 + p, b*RC + r] = x[c*RC + r, 128*b + p]
    Device, per 256-row chunk:
      stage A (weights-stationary): Y^T_b[i, r] = sum_p am[p,128b+i] xT_b[p,r]
        -> Y sbuf [p=f%128 ; (b, r)]
      corner turn: Z[32*pl+bb, RC*t+r] = Y[4*t+pl, RC*bb+r]  (32 plain DMAs)
      stage B: O^T_t[j, r] = sum_q bm[q,128t+j] Z_t[q, r]
        -> oT[c*128 + q', t*RC + r], host un-permutes.
    """
    import concourse.mybir as mybir

    nc = tc.nc
    f32 = mybir.dt.float32
    bf16 = mybir.dt.bfloat16

    consts = ctx.enter_context(tc.tile_pool(name="consts", bufs=1))
    xpool = ctx.enter_context(tc.tile_pool(name="xpool", bufs=3))
    ypool = ctx.enter_context(tc.tile_pool(name="ypool", bufs=2))
    zpool = ctx.enter_context(tc.tile_pool(name="zpool", bufs=2))
    opool = ctx.enter_context(tc.tile_pool(name="opool", bufs=2))
    psA = ctx.enter_context(tc.tile_pool(name="psA", bufs=3, space="PSUM"))
    psB = ctx.enter_context(tc.tile_pool(name="psB", bufs=3, space="PSUM"))

    am = consts.tile([128, DIM], bf16, tag="am")
    bm = consts.tile([128, DIM], bf16, tag="bm")
    for cc in range(4):
        nc.sync.dma_start(am[:, 1024 * cc:1024 * (cc + 1)],
                          amats[:, 1024 * cc:1024 * (cc + 1)])
    for cc in range(4):
        nc.sync.dma_start(bm[:, 1024 * cc:1024 * (cc + 1)],
                          bmats[:, 1024 * cc:1024 * (cc + 1)])

    # greedy DVE/ACT balance for PSUM->SBUF bf16 evacuations of [128, 2*RC]
    load = {"dve": 0.0, "act": 0.0}
    cost = {"dve": 392.0, "act": 357.0}

    def copy(dst, src):
        eng = min(("dve", "act"), key=lambda e: load[e] + cost[e])
        load[eng] += cost[eng]
        (nc.vector.tensor_copy if eng == "dve" else nc.scalar.copy)(dst, src)

    for c in range(NCHUNK):
        xin = xpool.tile([128, W3], bf16, tag="xin")
        nc.sync.dma_start(xin[:], xT[128 * c:128 * (c + 1), :])

        Y = ypool.tile([128, W3], bf16, tag="Y")
        for g in range(16):
            pt = psA.tile([128, 2 * RC], f32, tag="ptA")
            for j in range(2):
                b = 2 * g + j
                nc.tensor.matmul(
                    pt[:, RC * j:RC * (j + 1)],
                    am[:, 128 * b:128 * (b + 1)],
                    xin[:, RC * b:RC * (b + 1)],
                    start=True, stop=True)
            copy(Y[:, 2 * RC * g:2 * RC * (g + 1)], pt[:])

        Z = zpool.tile([128, W3], bf16, tag="Z")
        for t in range(32):
            eng = nc.sync if t % 2 == 0 else nc.scalar
            eng.dma_start(Z[:, RC * t:RC * (t + 1)], Y[4 * t:4 * t + 4, :])

        O = opool.tile([128, W3], bf16, tag="O")
        for g in range(16):
            pt = psB.tile([128, 2 * RC], f32, tag="ptB")
            for j in range(2):
                t = 2 * g + j
                nc.tensor.matmul(
                    pt[:, RC * j:RC * (j + 1)],
                    bm[:, 128 * t:128 * (t + 1)],
                    Z[:, RC * t:RC * (t + 1)],
                    start=True, stop=True)
            copy(O[:, 2 * RC * g:2 * RC * (g + 1)], pt[:])

        nc.sync.dma_start(oT[128 * c:128 * (c + 1), :], O[:])


def _emit_kernel_v4(ctx, tc, oT, xT, amats, bmats):
    """bf16 feature-major pipeline; corner turn on the DVE.

    Stage A's lhsT columns are sigma-permuted on the host so PSUM partition
    s = 32*pl + t holds feature 128*b + 4*t + pl.  The corner turn is then
    quadrant-local -- Z[32*pl+b ; t, r] = Y[32*pl+t ; b, r] -- which is
    exactly the DVE stream-transpose (32x32 blocks) applied per-r via
    strided views: in_ [s; r, b], out [q; r, t].
    """
    import concourse.mybir as mybir

    nc = tc.nc
    f32 = mybir.dt.float32
    bf16 = mybir.dt.bfloat16

    consts = ctx.enter_context(tc.tile_pool(name="consts", bufs=1))
    xpool = ctx.enter_context(tc.tile_pool(name="xpool", bufs=3))
    ypool = ctx.enter_context(tc.tile_pool(name="ypool", bufs=2))
    zpool = ctx.enter_context(tc.tile_pool(name="zpool", bufs=2))
    opool = ctx.enter_context(tc.tile_pool(name="opool", bufs=2))
    psA = ctx.enter_context(tc.tile_pool(name="psA", bufs=2, space="PSUM"))
    psB = ctx.enter_context(tc.tile_pool(name="psB", bufs=2, space="PSUM"))

    am = consts.tile([128, DIM], bf16, tag="am")
    bm = consts.tile([128, DIM], bf16, tag="bm")
    for cc in range(4):
        nc.sync.dma_start(am[:, 1024 * cc:1024 * (cc + 1)],
                          amats[:, 1024 * cc:1024 * (cc + 1)])
    for cc in range(4):
        nc.sync.dma_start(bm[:, 1024 * cc:1024 * (cc + 1)],
                          bmats[:, 1024 * cc:1024 * (cc + 1)])

    # greedy DVE/ACT balance for PSUM->SBUF bf16 evacuations of [128, 1024]
    # (GPSIMD cannot access PSUM -- BIR verifier rejects it)
    load = {"dve": 0.0, "act": 0.0}
    cost = {"dve": 1192.0, "act": 997.0}
    eng_op = {"dve": nc.vector.tensor_copy, "act": nc.scalar.copy}

    def copy(dst, src):
        eng = min(load, key=lambda e: load[e] + cost[e])
        load[eng] += cost[eng]
        eng_op[eng](dst, src)

    NTR = 4                  # corner-turn split (r-slices per chunk)
    RQ = RC // NTR

    for c in range(NCHUNK):
        xin = xpool.tile([128, W3], bf16, tag="xin")
        nc.sync.dma_start(xin[:], xT[128 * c:128 * (c + 1), :])

        Y = ypool.tile([128, W3], bf16, tag="Y")
        for g in range(8):
            pt = psA.tile([128, 1024], f32, tag="ptA")
            for j in range(4):
                b = 4 * g + j
                nc.tensor.matmul(
                    pt[:, RC * j:RC * (j + 1)],
                    am[:, 128 * b:128 * (b + 1)],
                    xin[:, RC * b:RC * (b + 1)],
                    start=True, stop=True)
            copy(Y[:, 1024 * g:1024 * (g + 1)], pt[:])

        Z = zpool.tile([128, W3], bf16, tag="Z")
        yv = Y[:].rearrange("s (b r) -> s b r", b=32, r=RC)
        zv = Z[:].rearrange("q (t r) -> q t r", t=32, r=RC)
        for q in range(NTR):
            inv = yv[:, :, RQ * q:RQ * (q + 1)].transpose([0, 2, 1])
            outv = zv[:, :, RQ * q:RQ * (q + 1)].transpose([0, 2, 1])
            nc.vector.transpose(outv, inv)
            load["dve"] += (58 + 32 * RQ) / 0.96

        O = opool.tile([128, W3], bf16, tag="O")
        for g in range(8):
            pt = psB.tile([128, 1024], f32, tag="ptB")
            for j in range(4):
                t = 4 * g + j
                nc.tensor.matmul(
                    pt[:, RC * j:RC * (j + 1)],
                    bm[:, 128 * t:128 * (t + 1)],
                    Z[:, RC * t:RC * (t + 1)],
                    start=True, stop=True)
            copy(O[:, 1024 * g:1024 * (g + 1)], pt[:])

        nc.sync.dma_start(oT[128 * c:128 * (c + 1), :], O[:])


def _emit_kernel_v5(ctx, tc, oT, xT, amats, bmats, ydram):
    """bf16 feature-major pipeline; corner turn via HBM round trip.

    The f~ relabeling (f~ = 32*s + b = 128*t + 32*pl + b, s = 4*t + pl) is
    exactly the row-major flattening of Y's (s, b) indices, so the Y->DRAM
    write per chunk is one PLAIN contiguous 2 MB transfer (full line rate,
    16-engine spread).  The read-back gathers Z[q; t, r] = ydram[128t+q, r]
    with a clean 3-dim AP (runs of RC elements, dst = 128 partitions).
    """
    import concourse.mybir as mybir

    nc = tc.nc
    f32 = mybir.dt.float32
    bf16 = mybir.dt.bfloat16

    consts = ctx.enter_context(tc.tile_pool(name="consts", bufs=1))
    xpool = ctx.enter_context(tc.tile_pool(name="xpool", bufs=NCHUNK))
    ypool = ctx.enter_context(tc.tile_pool(name="ypool", bufs=2))
    zpool = ctx.enter_context(tc.tile_pool(name="zpool", bufs=2))
    opool = ctx.enter_context(tc.tile_pool(name="opool", bufs=2))
    psA = ctx.enter_context(tc.tile_pool(name="psA", bufs=2, space="PSUM"))
    psB = ctx.enter_context(tc.tile_pool(name="psB", bufs=2, space="PSUM"))

    am = consts.tile([128, DIM], bf16, tag="am")
    bm = consts.tile([128, DIM], bf16, tag="bm")

    # all xin loads issued up front on SP so no later blocking wait on the
    # SP queue can delay them; chunk 0 split fine + interleaved with am so
    # the first stage-A matmuls start as early as possible
    xins = [xpool.tile([128, W3], bf16, tag="xin", name=f"xin{c}")
            for c in range(NCHUNK)]
    for g in range(4):
        nc.sync.dma_start(xins[0][:, 2048 * g:2048 * (g + 1)],
                          xT[0:128, 2048 * g:2048 * (g + 1)])
        nc.sync.dma_start(am[:, 1024 * g:1024 * (g + 1)],
                          amats[:, 1024 * g:1024 * (g + 1)])
    for c in range(1, NCHUNK):
        nc.sync.dma_start(xins[c][:], xT[128 * c:128 * (c + 1), :])
    for cc in range(4):
        nc.sync.dma_start(bm[:, 1024 * cc:1024 * (cc + 1)],
                          bmats[:, 1024 * cc:1024 * (cc + 1)])

    load = {"dve": 0.0, "act": 0.0}
    cost = {"dve": 1192.0, "act": 997.0}
    eng_op = {"dve": nc.vector.tensor_copy, "act": nc.scalar.copy}

    def copy(dst, src):
        eng = min(load, key=lambda e: load[e] + cost[e])
        load[eng] += cost[eng]
        eng_op[eng](dst, src)

    # ydram: [NCHUNK * 4096, RC] bf16; chunk c rows [4096c, 4096(c+1))
    for c in range(NCHUNK):
        xin = xins[c]
        Y = ypool.tile([128, W3], bf16, tag="Y")
        for g in range(8):
            pt = psA.tile([128, 1024], f32, tag="ptA")
            for j in range(4):
                b = 4 * g + j
                nc.tensor.matmul(
                    pt[:, RC * j:RC * (j + 1)],
                    am[:, 128 * b:128 * (b + 1)],
                    xin[:, RC * b:RC * (b + 1)],
                    start=True, stop=True)
            copy(Y[:, 1024 * g:1024 * (g + 1)], pt[:])
            if g % 4 == 3:
                # corner turn hop 1 (half): dram rows 32 s + b for the 16
                # b's evacuated so far; issued on ACT after its same-queue
                # evac deps, so the wait never head-of-line-blocks SP
                h = g // 4
                ywr = ydram[4096 * c:4096 * (c + 1), :].rearrange(
                    "(s b) r -> s b r", s=128, b=32)[:, 16 * h:16 * (h + 1)]
                nc.scalar.dma_start(
                    ywr, Y[:, 4096 * h:4096 * (h + 1)].rearrange(
                        "s (b r) -> s b r", b=16))

        # hop 2 on SP, split by t for stage-B overlap
        Z = zpool.tile([128, W3], bf16, tag="Z")
        zv = Z[:].rearrange("q (t r) -> q t r", t=32)
        zrd = ydram[4096 * c:4096 * (c + 1), :].rearrange(
            "(t q) r -> q t r", t=32, q=128)
        for h in range(2):
            nc.sync.dma_start(zv[:, 16 * h:16 * (h + 1)],
                              zrd[:, 16 * h:16 * (h + 1)])

        O = opool.tile([128, W3], bf16, tag="O")
        for g in range(8):
            pt = psB.tile([128, 1024], f32, tag="ptB")
            for j in range(4):
                t = 4 * g + j
                nc.tensor.matmul(
                    pt[:, RC * j:RC * (j + 1)],
                    bm[:, 128 * t:128 * (t + 1)],
                    Z[:, RC * t:RC * (t + 1)],
                    start=True, stop=True)
            copy(O[:, 1024 * g:1024 * (g + 1)], pt[:])
            if g % 4 == 3:
                h = g // 4
                nc.sync.dma_start(
                    oT[128 * c:128 * (c + 1), 4096 * h:4096 * (h + 1)],
                    O[:, 4096 * h:4096 * (h + 1)])


def _hoist_matmul_waits(nc):
    """Walrus's fp32/transpose matmul (self-loading LDWEIGHTS) accepts fewer
    sync waits than Tile may assign. Hoist multi-waits onto a PE NoOp inserted
    just before the matmul — same engine queue, so ordering is identical."""
    import concourse.mybir as mybir

    n_hoisted = 0
    for blk in nc.m.functions[0].blocks:
        il = blk.instructions
        i = 0
        while i < len(il):
            inst = il[i]
            si = inst.sync_info
            if (si is not None and len(si.on_wait) > 1
                    and not isinstance(inst, mybir.InstNoOp)):
                waits = list(si.on_wait)
                # keep the last wait on the matmul; one NoOp per extra wait
                # (cayman instructions carry at most one sem-wait each)
                for k, w in enumerate(waits[:-1]):
                    nop = mybir.InstNoOp(
                        name=f"{inst.name}_hw{k}", engine=inst.engine,
                        bass_nofuse=True)
                    nop.sync_info = mybir.SyncInfo(on_wait=[w], on_update=[])
                    nc.register_instruction(nop, overwrite=True)
                    il.insert(i, nop)
                    i += 1
                    n_hoisted += 1
                inst.sync_info = mybir.SyncInfo(
                    on_wait=[waits[-1]], on_update=list(si.on_update))
            i += 1
    return n_hoisted


_CACHED = {}
VARIANT = "v5"   # v1 fp32 | v2 f32r | v3 bf16+DMA turn | v4 bf16+DVE turn


def _build_bass(variant=None):
    variant = variant or VARIANT
    if variant in _CACHED:
        return _CACHED[variant]
    from contextlib import ExitStack
    import concourse.bass as bass
    import concourse.tile as tile
    import concourse.mybir as mybir

    f32 = mybir.dt.float32
    bf16 = mybir.dt.bfloat16
    nc = bass.Bass("TRN2", target_bir_lowering=False, debug=False,
                   num_devices=NCORES)
    if variant in ("v3", "v4", "v5"):
        xT = nc.dram_tensor("xT", [NCHUNK * 128, W3], bf16,
                            kind="ExternalInput").ap()
        amats = nc.dram_tensor("amats", [128, DIM], bf16,
                               kind="ExternalInput").ap()
        bmats = nc.dram_tensor("bmats", [128, DIM], bf16,
                               kind="ExternalInput").ap()
        oT = nc.dram_tensor("oT", [NCHUNK * 128, W3], bf16,
                            kind="ExternalOutput").ap()
        if variant == "v5":
            ydram = nc.dram_tensor("ydram", [NCHUNK * DIM, RC], bf16,
                                   kind="Internal").ap()
            with tile.TileContext(nc) as tc:
                with ExitStack() as ctx:
                    _emit_kernel_v5(ctx, tc, oT, xT, amats, bmats, ydram)
        else:
            emit3 = _emit_kernel_v3 if variant == "v3" else _emit_kernel_v4
            with tile.TileContext(nc) as tc:
                with ExitStack() as ctx:
                    emit3(ctx, tc, oT, xT, amats, bmats)
    else:
        x = nc.dram_tensor("x", [RPC, DIM], f32, kind="ExternalInput").ap()
        amats = nc.dram_tensor("amats", [128, DIM], f32,
                               kind="ExternalInput").ap()
        bmats = nc.dram_tensor("bmats", [128, DIM], f32,
                               kind="ExternalInput").ap()
        ident = nc.dram_tensor("ident", [128, 128], f32,
                               kind="ExternalInput").ap()
        out = nc.dram_tensor("out", [RPC, DIM], f32, kind="ExternalOutput").ap()

        emit = _emit_kernel if variant == "v1" else _emit_kernel_v2
        with tile.TileContext(nc) as tc:
            with ExitStack() as ctx:
                emit(ctx, tc, out, x, amats, bmats, ident)

    _hoist_matmul_waits(nc)
    _CACHED[variant] = nc
    return nc


def make_in_maps(x, angles):
    x = np.ascontiguousarray(np.asarray(x, np.float32))
    amats, bmats = _build_mats(angles)
    ident = np.eye(128, dtype=np.float32)
    return [
        {"x": x[c * RPC:(c + 1) * RPC], "amats": amats, "bmats": bmats,
         "ident": ident}
        for c in range(NCORES)
    ]


def make_in_maps_v3(x, angles, sigma=False):
    import ml_dtypes
    bf = ml_dtypes.bfloat16
    amats, bmats = _build_mats(angles)
    if sigma:
        # v4: PSUM partition s of block b holds feature 4*(s%32) + s//32
        perm = np.array([4 * (s % 32) + s // 32 for s in range(128)])
        amats = np.ascontiguousarray(
            amats.reshape(128, 32, 128)[:, :, perm].reshape(128, DIM))
    amb = np.ascontiguousarray(amats.astype(bf))
    bmb = np.ascontiguousarray(bmats.astype(bf))
    x = np.asarray(x, np.float32)
    maps = []
    for c in range(NCORES):
        xc = x[c * RPC:(c + 1) * RPC].astype(bf)        # [RPC, DIM]
        # xT[ch*128 + p, b*RC + r] = xc[ch*RC + r, 128*b + p]
        xp = xc.reshape(NCHUNK, RC, 32, 128).transpose(0, 3, 2, 1)
        xp = np.ascontiguousarray(xp).reshape(NCHUNK * 128, W3)
        maps.append({"xT": xp, "amats": amb, "bmats": bmb})
    return maps


def _unpack_out_v3(oT):
    """oT [NCHUNK*128, W3] bf16 -> [RPC, DIM] f32 in natural order."""
    arr = np.asarray(oT).reshape(NCHUNK, 4, 32, 32, RC)   # [c, pl, b', t, r]
    arr = arr.transpose(0, 4, 2, 3, 1)                    # [c, r, b', t, pl]
    return np.ascontiguousarray(arr).reshape(RPC, DIM).astype(np.float32)


def run_on_hw(x, angles, trace=False, trace_kwargs=None, variant=None):
    from concourse.bass_utils import run_bass_kernel_spmd
    variant = variant or VARIANT
    nc = _build_bass(variant)
    if variant in ("v3", "v4", "v5"):
        in_maps = make_in_maps_v3(x, angles, sigma=(variant == "v4"))
    else:
        in_maps = make_in_maps(x, angles)
    res = run_bass_kernel_spmd(
        nc, in_maps, core_ids=list(range(NCORES)), trace=trace,
        **(trace_kwargs or {}))
    if variant in ("v3", "v4", "v5"):
        out = np.concatenate(
            [_unpack_out_v3(res.results[c]["oT"]) for c in range(NCORES)],
            axis=0)
    else:
        out = np.concatenate(
            [res.results[c]["out"] for c in range(NCORES)], axis=0)
    return out, res


def kernel(x, angles):
    last_err = None
    for attempt in range(3):
        try:
            out, _ = run_on_hw(x, angles, trace=False)
            return np.ascontiguousarray(out.astype(np.float32))
        except Exception as e:  # transient NRT/device errors: retry
            last_err = e
            import time
            time.sleep(5)
    raise last_err

